# revision 1
# baseline (speedup 1.0000x reference)
"""Cross-modal attention kernel for Trainium2 (Bass/Tile), data-parallel over
batch across 8 NeuronCores.

Math (per batch sample, N = 64*64 = 4096, D = 128):
    q = (s*Wq) @ cape + s*bq          [D, N]   (s = D**-0.5 folded into Wq,bq)
    k = Wk @ era5                     [D, N]   (bk dropped: constant along the
                                               softmax axis, cancels)
    S^T = k^T q                       [N, N]   computed in [128kk x 128qq] tiles
    P = exp(S^T)                      softmax numerator, kk on partitions
    U = (Wo@Wv @ era5) @ P            [128, N] Wo folded into V; softmax
                                               denominator = ones-column of the
                                               rhs -> column 128 of the output
    out = U[:, :128]/denom + (Wo@bv + bo)

Normalization is deferred past the value/output projections (both linear per
query column), so no per-element multiply over the NxN attention matrix is
ever needed; the denominator rides along as a 129th matmul output column.
"""

import os
import numpy as np
from contextlib import ExitStack

import concourse.bass as bass
import concourse.bacc as bacc
import concourse.mybir as mybir
import concourse.tile as tile
from concourse.bass_utils import run_bass_kernel_spmd
import ml_dtypes

AFT = mybir.ActivationFunctionType
BF16 = mybir.dt.bfloat16
F32 = mybir.dt.float32

N = 4096          # h*w
D = 128           # attn dim == cape channels
NCORES = 8
NKC = N // 128    # 32 kk chunks of 128
NQB = N // 128    # 32 qq blocks of 128
GROUPS = (12, 12, 8)   # kk chunks per exp group (3+3+2 PSUM banks pattern)
VSTride = 136     # free-dim stride of one v'T chunk in SBUF (128 data + ones + pad)

_CACHE = {}
LAST_RESULTS = None


def build_program():
    nc = bacc.Bacc("TRN2", debug=False, target_bir_lowering=False)

    cape = nc.dram_tensor("cape", [128, N], BF16, kind="ExternalInput")
    era5a = nc.dram_tensor("era5a", [128, N], BF16, kind="ExternalInput")
    era5b = nc.dram_tensor("era5b", [128, N], BF16, kind="ExternalInput")
    wq_t = nc.dram_tensor("wq_t", [128, 128], BF16, kind="ExternalInput")
    wk_t0 = nc.dram_tensor("wk_t0", [128, 128], BF16, kind="ExternalInput")
    wk_t1 = nc.dram_tensor("wk_t1", [128, 128], BF16, kind="ExternalInput")
    wp_t0 = nc.dram_tensor("wp_t0", [128, 128], BF16, kind="ExternalInput")
    wp_t1 = nc.dram_tensor("wp_t1", [128, 128], BF16, kind="ExternalInput")
    bq_d = nc.dram_tensor("bq", [128, 1], F32, kind="ExternalInput")
    bp_d = nc.dram_tensor("bp", [128, 1], F32, kind="ExternalInput")
    ident_d = nc.dram_tensor("ident", [128, 128], F32, kind="ExternalInput")
    out_d = nc.dram_tensor("out", [128, N], F32, kind="ExternalOutput")

    with tile.TileContext(nc) as tc, ExitStack() as ctx:
        consts = ctx.enter_context(tc.tile_pool(name="consts", bufs=1))
        big = ctx.enter_context(tc.tile_pool(name="big", bufs=1))
        ppool = ctx.enter_context(tc.tile_pool(name="pn", bufs=3))
        opool = ctx.enter_context(tc.tile_pool(name="small", bufs=2))
        ps_s = ctx.enter_context(tc.tile_pool(name="ps_s", bufs=2, space="PSUM"))
        ps_o = ctx.enter_context(tc.tile_pool(name="ps_o", bufs=2, space="PSUM"))

        # ---- constants / weights to SBUF ----
        wq_sb = consts.tile([128, 128], BF16, tag="wq")
        wk0_sb = consts.tile([128, 128], BF16, tag="wk0")
        wk1_sb = consts.tile([128, 128], BF16, tag="wk1")
        wp0_sb = consts.tile([128, 128], BF16, tag="wp0")
        wp1_sb = consts.tile([128, 128], BF16, tag="wp1")
        bq_sb = consts.tile([128, 1], F32, tag="bq")
        bp_sb = consts.tile([128, 1], F32, tag="bp")
        id_sb = consts.tile([128, 128], F32, tag="ident")
        nc.sync.dma_start(wq_sb[:], wq_t[:])
        nc.sync.dma_start(wk0_sb[:], wk_t0[:])
        nc.sync.dma_start(wk1_sb[:], wk_t1[:])
        nc.sync.dma_start(wp0_sb[:], wp_t0[:])
        nc.sync.dma_start(wp1_sb[:], wp_t1[:])
        nc.sync.dma_start(bq_sb[:], bq_d[:])
        nc.sync.dma_start(bp_sb[:], bp_d[:])
        nc.sync.dma_start(id_sb[:], ident_d[:])

        cape_sb = big.tile([128, N], BF16, tag="cape")
        era5a_sb = big.tile([128, N], BF16, tag="era5a")
        era5b_sb = big.tile([128, N], BF16, tag="era5b")
        nc.sync.dma_start(cape_sb[:], cape[:])
        nc.sync.dma_start(era5a_sb[:], era5a[:])
        nc.sync.dma_start(era5b_sb[:], era5b[:])

        q_sb = big.tile([128, N], BF16, tag="q")
        k_sb = big.tile([128, N], BF16, tag="k")
        vT_sb = big.tile([128, NKC * VSTride], BF16, tag="vT")

        # ---- projections ----
        # q = (s Wq) @ cape + s bq     [D, N]
        for j in range(8):
            sl = slice(j * 512, (j + 1) * 512)
            pq = ps_s.tile([128, 512], F32, tag="s")
            nc.tensor.matmul(pq[:], wq_sb[:], cape_sb[:, sl])
            nc.vector.tensor_scalar_add(q_sb[:, sl], pq[:], bq_sb[:])
        # k = Wk @ era5                [D, N]
        for j in range(8):
            sl = slice(j * 512, (j + 1) * 512)
            pk = ps_s.tile([128, 512], F32, tag="s")
            nc.tensor.matmul(pk[:], wk0_sb[:], era5a_sb[:, sl], start=True, stop=False)
            nc.tensor.matmul(pk[:], wk1_sb[:], era5b_sb[:, sl], start=False, stop=True)
            nc.vector.tensor_copy(k_sb[:, sl], pk[:])
        # v'T chunks: v'T[kk, d] = era5^T @ (Wo Wv)^T, chunk kk of 128
        for c4 in range(NKC // 4):
            pv = ps_s.tile([128, 512], F32, tag="s")
            for i in range(4):
                c = c4 * 4 + i
                ksl = slice(c * 128, (c + 1) * 128)
                osl = slice(i * 128, (i + 1) * 128)
                nc.tensor.matmul(pv[:, osl], era5a_sb[:, ksl], wp0_sb[:],
                                 start=True, stop=False)
                nc.tensor.matmul(pv[:, osl], era5b_sb[:, ksl], wp1_sb[:],
                                 start=False, stop=True)
            for i in range(4):
                c = c4 * 4 + i
                nc.vector.tensor_copy(
                    vT_sb[:, c * VSTride:c * VSTride + 128],
                    pv[:, i * 128:(i + 1) * 128])
        # ones column (softmax denominator) per v'T chunk
        ones_view = vT_sb.rearrange("p (c x) -> p c x", x=VSTride)[:, :, 128:129]
        nc.gpsimd.memset(ones_view, 1.0)

        # ---- main attention loop over query blocks ----
        for b in range(NQB):
            qsl = slice(b * 128, (b + 1) * 128)
            o_tile = ps_o.tile([128, 129], F32, tag="o")
            c0 = 0
            for G in GROUPS:
                s_tile = ps_s.tile([128, G * 128], F32, tag="s")
                p_tile = ppool.tile([128, G * 128], BF16, tag="p")
                for c in range(c0, c0 + G):
                    osl = slice((c - c0) * 128, (c - c0 + 1) * 128)
                    nc.tensor.matmul(s_tile[:, osl],
                                     k_sb[:, c * 128:(c + 1) * 128],
                                     q_sb[:, qsl])
                nc.scalar.activation(p_tile[:], s_tile[:], AFT.Exp)
                for c in range(c0, c0 + G):
                    osl = slice((c - c0) * 128, (c - c0 + 1) * 128)
                    nc.tensor.matmul(o_tile[:],
                                     p_tile[:, osl],
                                     vT_sb[:, c * VSTride:c * VSTride + 129],
                                     start=(c == 0), stop=(c == NKC - 1))
                c0 += G

            recip_t = opool.tile([128, 1], F32, tag="recip")
            nc.vector.reciprocal(recip_t[:], o_tile[:, 128:129])
            nrm_t = opool.tile([128, 128], F32, tag="nrm")
            nc.vector.tensor_scalar_mul(nrm_t[:], o_tile[:, 0:128], recip_t[:])
            t_tile = ps_o.tile([128, 128], F32, tag="o")
            nc.tensor.transpose(t_tile[:], nrm_t[:], id_sb[:])
            outb_t = opool.tile([128, 128], F32, tag="outb")
            nc.vector.tensor_scalar_add(outb_t[:], t_tile[:], bp_sb[:])
            nc.sync.dma_start(out_d[:, qsl], outb_t[:])

    nc.compile()
    return nc


def _get_program():
    if "nc" not in _CACHE:
        _CACHE["nc"] = build_program()
    return _CACHE["nc"]


def kernel(cape_features, era5_features, Wq, bq, Wk, bk, Wv, bv, Wo, bo):
    global LAST_RESULTS
    bf = ml_dtypes.bfloat16
    cape = np.asarray(cape_features, np.float32)
    era5 = np.asarray(era5_features, np.float32)
    Wq = np.asarray(Wq, np.float32)
    bq = np.asarray(bq, np.float32)
    Wk = np.asarray(Wk, np.float32)
    Wv = np.asarray(Wv, np.float32)
    bv = np.asarray(bv, np.float32)
    Wo = np.asarray(Wo, np.float32)
    bo = np.asarray(bo, np.float32)

    B = cape.shape[0]
    scale = np.float32(Wq.shape[0] ** -0.5)

    wq_t = np.ascontiguousarray((Wq * scale).T).astype(bf)       # [Cc, D]
    wk_t = np.ascontiguousarray(Wk.T)                            # [Ce, D]
    Wp = Wo @ Wv                                                 # [Cc, Ce]
    wp_t = np.ascontiguousarray(Wp.T)                            # [Ce, Cc]
    bq_e = np.ascontiguousarray((bq * scale).reshape(128, 1), dtype=np.float32)
    bp_e = np.ascontiguousarray((Wo @ bv + bo).reshape(128, 1), dtype=np.float32)
    ident = np.eye(128, dtype=np.float32)

    common = {
        "wq_t": wq_t,
        "wk_t0": wk_t[:128].astype(bf), "wk_t1": wk_t[128:].astype(bf),
        "wp_t0": wp_t[:128].astype(bf), "wp_t1": wp_t[128:].astype(bf),
        "bq": bq_e, "bp": bp_e, "ident": ident,
    }
    in_maps = []
    for s in range(B):
        e = era5[s].reshape(256, N)
        in_maps.append(dict(common,
                            cape=cape[s].reshape(128, N).astype(bf),
                            era5a=e[:128].astype(bf),
                            era5b=e[128:].astype(bf)))

    nc = _get_program()
    res = run_bass_kernel_spmd(
        nc, in_maps, core_ids=list(range(NCORES)),
        trace=bool(int(os.environ.get("KBENCH_TRACE", "0"))),
    )
    LAST_RESULTS = res
    out = np.stack([res.results[s]["out"].reshape(128, 64, 64) for s in range(B)])
    return out.astype(np.float32)


# revision 6
# speedup vs baseline: 1.0000x; 1.0000x over previous
"""Cross-modal attention kernel for Trainium2 (Bass/Tile), data-parallel over
batch across 8 NeuronCores.

Math (per batch sample, N = 64*64 = 4096, D = 128):
    q = (s*Wq) @ cape + s*bq          [D, N]   (s = D**-0.5 folded into Wq,bq)
    k = Wk @ era5                     [D, N]   (bk dropped: constant along the
                                               softmax axis, cancels)
    S^T = k^T q                       [N, N]   computed in [128kk x 128qq] tiles
    P = exp(S^T)                      softmax numerator, kk on partitions
    U = (Wo@Wv @ era5) @ P            [128, N] Wo folded into V; softmax
                                               denominator = ones-column of the
                                               rhs -> column 128 of the output
    out = U[:, :128]/denom + (Wo@bv + bo)

Normalization is deferred past the value/output projections (both linear per
query column), so no per-element multiply over the NxN attention matrix is
ever needed; the denominator rides along as a 129th matmul output column.
"""

import os
import numpy as np
from contextlib import ExitStack

import concourse.bass as bass
import concourse.bacc as bacc
import concourse.mybir as mybir
import concourse.tile as tile
from concourse.bass_utils import run_bass_kernel_spmd
import ml_dtypes

AFT = mybir.ActivationFunctionType
BF16 = mybir.dt.bfloat16
F32 = mybir.dt.float32

N = 4096          # h*w
D = 128           # attn dim == cape channels
NCORES = 8
NKC = N // 128    # 32 kk chunks of 128
NQB = N // 128    # 32 qq blocks of 128
GROUPS = (12, 12, 8)   # kk chunks per exp group (3+3+2 PSUM banks pattern)
VSTride = 136     # free-dim stride of one v'T chunk in SBUF (128 data + ones + pad)

_CACHE = {}
LAST_RESULTS = None


def build_program():
    nc = bacc.Bacc("TRN2", debug=False, target_bir_lowering=False)

    cape = nc.dram_tensor("cape", [128, N], BF16, kind="ExternalInput")
    era5a = nc.dram_tensor("era5a", [128, N], BF16, kind="ExternalInput")
    era5b = nc.dram_tensor("era5b", [128, N], BF16, kind="ExternalInput")
    wq_t = nc.dram_tensor("wq_t", [128, 128], BF16, kind="ExternalInput")
    wk_t0 = nc.dram_tensor("wk_t0", [128, 128], BF16, kind="ExternalInput")
    wk_t1 = nc.dram_tensor("wk_t1", [128, 128], BF16, kind="ExternalInput")
    wp_t0 = nc.dram_tensor("wp_t0", [128, 128], BF16, kind="ExternalInput")
    wp_t1 = nc.dram_tensor("wp_t1", [128, 128], BF16, kind="ExternalInput")
    bq_d = nc.dram_tensor("bq", [128, 1], F32, kind="ExternalInput")
    # output is stored TRANSPOSED: [N, 128] = (out + bias)^T without bias;
    # host adds the (folded) bias and transposes back.
    out_d = nc.dram_tensor("out", [N, 128], F32, kind="ExternalOutput")

    with tile.TileContext(nc) as tc, ExitStack() as ctx:
        consts = ctx.enter_context(tc.tile_pool(name="consts", bufs=1))
        big = ctx.enter_context(tc.tile_pool(name="big", bufs=1))
        ppool = ctx.enter_context(tc.tile_pool(name="pn", bufs=3))
        opool = ctx.enter_context(tc.tile_pool(name="small", bufs=2))
        ps_s = ctx.enter_context(tc.tile_pool(name="ps_s", bufs=2, space="PSUM"))
        ps_o = ctx.enter_context(tc.tile_pool(name="ps_o", bufs=2, space="PSUM"))

        # ---- constants / weights to SBUF ----
        wq_sb = consts.tile([128, 128], BF16, tag="wq")
        wk0_sb = consts.tile([128, 128], BF16, tag="wk0")
        wk1_sb = consts.tile([128, 128], BF16, tag="wk1")
        wp0_sb = consts.tile([128, 128], BF16, tag="wp0")
        wp1_sb = consts.tile([128, 128], BF16, tag="wp1")
        bq_sb = consts.tile([128, 1], F32, tag="bq")
        nc.sync.dma_start(wq_sb[:], wq_t[:])
        nc.sync.dma_start(wk0_sb[:], wk_t0[:])
        nc.sync.dma_start(wk1_sb[:], wk_t1[:])
        nc.sync.dma_start(wp0_sb[:], wp_t0[:])
        nc.sync.dma_start(wp1_sb[:], wp_t1[:])
        nc.sync.dma_start(bq_sb[:], bq_d[:])

        cape_sb = big.tile([128, N], BF16, tag="cape")
        era5a_sb = big.tile([128, N], BF16, tag="era5a")
        era5b_sb = big.tile([128, N], BF16, tag="era5b")
        nc.sync.dma_start(cape_sb[:], cape[:])
        nc.sync.dma_start(era5a_sb[:], era5a[:])
        nc.sync.dma_start(era5b_sb[:], era5b[:])

        q_sb = big.tile([128, N], BF16, tag="q")
        k_sb = big.tile([128, N], BF16, tag="k")
        vT_sb = big.tile([128, NKC * VSTride], BF16, tag="vT")

        # ---- projections ----
        # q = (s Wq) @ cape + s bq     [D, N]
        for j in range(8):
            sl = slice(j * 512, (j + 1) * 512)
            pq = ps_s.tile([128, 512], F32, tag="s")
            nc.tensor.matmul(pq[:], wq_sb[:], cape_sb[:, sl])
            nc.vector.tensor_scalar_add(q_sb[:, sl], pq[:], bq_sb[:])
        # k = Wk @ era5                [D, N]
        for j in range(8):
            sl = slice(j * 512, (j + 1) * 512)
            pk = ps_s.tile([128, 512], F32, tag="s")
            nc.tensor.matmul(pk[:], wk0_sb[:], era5a_sb[:, sl], start=True, stop=False)
            nc.tensor.matmul(pk[:], wk1_sb[:], era5b_sb[:, sl], start=False, stop=True)
            nc.vector.tensor_copy(k_sb[:, sl], pk[:])
        # v'T chunks: v'T[kk, d] = era5^T @ (Wo Wv)^T, chunk kk of 128
        for c4 in range(NKC // 4):
            pv = ps_s.tile([128, 512], F32, tag="s")
            for i in range(4):
                c = c4 * 4 + i
                ksl = slice(c * 128, (c + 1) * 128)
                osl = slice(i * 128, (i + 1) * 128)
                nc.tensor.matmul(pv[:, osl], era5a_sb[:, ksl], wp0_sb[:],
                                 start=True, stop=False)
                nc.tensor.matmul(pv[:, osl], era5b_sb[:, ksl], wp1_sb[:],
                                 start=False, stop=True)
            for i in range(4):
                c = c4 * 4 + i
                nc.vector.tensor_copy(
                    vT_sb[:, c * VSTride:c * VSTride + 128],
                    pv[:, i * 128:(i + 1) * 128])
        # ones column (softmax denominator) per v'T chunk
        ones_view = vT_sb.rearrange("p (c x) -> p c x", x=VSTride)[:, :, 128:129]
        nc.gpsimd.memset(ones_view, 1.0)

        # ---- main attention loop over query blocks ----
        for b in range(NQB):
            qsl = slice(b * 128, (b + 1) * 128)
            o_tile = ps_o.tile([128, 129], F32, tag="o")
            c0 = 0
            for G in GROUPS:
                s_tile = ps_s.tile([128, G * 128], F32, tag="s")
                p_tile = ppool.tile([128, G * 128], BF16, tag="p")
                for c in range(c0, c0 + G):
                    osl = slice((c - c0) * 128, (c - c0 + 1) * 128)
                    nc.tensor.matmul(s_tile[:, osl],
                                     k_sb[:, c * 128:(c + 1) * 128],
                                     q_sb[:, qsl])
                nc.scalar.activation(p_tile[:], s_tile[:], AFT.Exp)
                for c in range(c0, c0 + G):
                    osl = slice((c - c0) * 128, (c - c0 + 1) * 128)
                    nc.tensor.matmul(o_tile[:],
                                     p_tile[:, osl],
                                     vT_sb[:, c * VSTride:c * VSTride + 129],
                                     start=(c == 0), stop=(c == NKC - 1))
                c0 += G

            recip_t = opool.tile([128, 1], F32, tag="recip")
            nc.vector.reciprocal(recip_t[:], o_tile[:, 128:129])
            nrm_t = opool.tile([128, 128], F32, tag="nrm")
            nc.vector.tensor_scalar_mul(nrm_t[:], o_tile[:, 0:128], recip_t[:])
            nc.sync.dma_start(out_d[qsl, :], nrm_t[:])

    nc.compile()
    return nc


def _get_program():
    if "nc" not in _CACHE:
        _CACHE["nc"] = build_program()
    return _CACHE["nc"]


def kernel(cape_features, era5_features, Wq, bq, Wk, bk, Wv, bv, Wo, bo):
    global LAST_RESULTS
    bf = ml_dtypes.bfloat16
    cape = np.asarray(cape_features, np.float32)
    era5 = np.asarray(era5_features, np.float32)
    Wq = np.asarray(Wq, np.float32)
    bq = np.asarray(bq, np.float32)
    Wk = np.asarray(Wk, np.float32)
    Wv = np.asarray(Wv, np.float32)
    bv = np.asarray(bv, np.float32)
    Wo = np.asarray(Wo, np.float32)
    bo = np.asarray(bo, np.float32)

    B = cape.shape[0]
    scale = np.float32(Wq.shape[0] ** -0.5)

    wq_t = np.ascontiguousarray((Wq * scale).T).astype(bf)       # [Cc, D]
    wk_t = np.ascontiguousarray(Wk.T)                            # [Ce, D]
    Wp = Wo @ Wv                                                 # [Cc, Ce]
    wp_t = np.ascontiguousarray(Wp.T)                            # [Ce, Cc]
    bq_e = np.ascontiguousarray((bq * scale).reshape(128, 1), dtype=np.float32)
    bp_e = (Wo @ bv + bo).astype(np.float32)          # added host-side

    common = {
        "wq_t": wq_t,
        "wk_t0": wk_t[:128].astype(bf), "wk_t1": wk_t[128:].astype(bf),
        "wp_t0": wp_t[:128].astype(bf), "wp_t1": wp_t[128:].astype(bf),
        "bq": bq_e,
    }
    in_maps = []
    for s in range(B):
        e = era5[s].reshape(256, N)
        in_maps.append(dict(common,
                            cape=cape[s].reshape(128, N).astype(bf),
                            era5a=e[:128].astype(bf),
                            era5b=e[128:].astype(bf)))

    nc = _get_program()
    res = run_bass_kernel_spmd(
        nc, in_maps, core_ids=list(range(NCORES)),
        trace=bool(int(os.environ.get("KBENCH_TRACE", "0"))),
    )
    LAST_RESULTS = res
    out = np.stack([
        (res.results[s]["out"].T + bp_e[:, None]).reshape(128, 64, 64)
        for s in range(B)
    ])
    return np.ascontiguousarray(out, dtype=np.float32)


# revision 10
# speedup vs baseline: 1.1920x; 1.1920x over previous
"""Cross-modal attention kernel for Trainium2 (Bass/Tile), data-parallel over
batch across 8 NeuronCores.

Math (per batch sample, N = 64*64 = 4096, D = 128):
    q = (s*Wq) @ cape + s*bq          [D, N]   (s = D**-0.5 folded into Wq,bq)
    k = Wk @ era5                     [D, N]   (bk dropped: constant along the
                                               softmax axis, cancels)
    S^T = k^T q                       [N, N]   computed in [128kk x 128qq] tiles
    P = exp(S^T)                      softmax numerator, kk on partitions
    U = (Wo@Wv @ era5) @ P            [128, N] Wo folded into V; softmax
                                               denominator = ones-column of the
                                               rhs -> column 128 of the output
    out = U[:, :128]/denom + (Wo@bv + bo)

Normalization is deferred past the value/output projections (both linear per
query column), so no per-element multiply over the NxN attention matrix is
ever needed; the denominator rides along as a 129th matmul output column.
"""

import os
import numpy as np
from contextlib import ExitStack

import concourse.bass as bass
import concourse.bacc as bacc
import concourse.mybir as mybir
import concourse.tile as tile
from concourse.bass_utils import run_bass_kernel_spmd
import ml_dtypes

AFT = mybir.ActivationFunctionType
BF16 = mybir.dt.bfloat16
F32 = mybir.dt.float32

N = 4096          # h*w
D = 128           # attn dim == cape channels
NCORES = 8
NKC = N // 128    # 32 kk chunks of 128
NSB = N // 512    # 8 query superblocks of 512
GROUPS = (3, 3, 3, 3, 3, 3, 3, 3, 3, 3, 2)   # kk chunks per exp group
VSTride = 136     # free-dim stride of one v'T chunk in SBUF (128 data + ones + pad)

_CACHE = {}
LAST_RESULTS = None


def build_program():
    nc = bacc.Bacc("TRN2", debug=False, target_bir_lowering=False)

    cape = nc.dram_tensor("cape", [128, N], BF16, kind="ExternalInput")
    era5a = nc.dram_tensor("era5a", [128, N], BF16, kind="ExternalInput")
    era5b = nc.dram_tensor("era5b", [128, N], BF16, kind="ExternalInput")
    wq_t = nc.dram_tensor("wq_t", [128, 128], BF16, kind="ExternalInput")
    wk_t0 = nc.dram_tensor("wk_t0", [128, 128], BF16, kind="ExternalInput")
    wk_t1 = nc.dram_tensor("wk_t1", [128, 128], BF16, kind="ExternalInput")
    wp_t0 = nc.dram_tensor("wp_t0", [128, 128], BF16, kind="ExternalInput")
    wp_t1 = nc.dram_tensor("wp_t1", [128, 128], BF16, kind="ExternalInput")
    bq_d = nc.dram_tensor("bq", [128, 1], F32, kind="ExternalInput")
    # output is stored TRANSPOSED: [N, 128] = (out + bias)^T without bias;
    # host adds the (folded) bias and transposes back.
    out_d = nc.dram_tensor("out", [N, 128], F32, kind="ExternalOutput")

    with tile.TileContext(nc) as tc, ExitStack() as ctx:
        consts = ctx.enter_context(tc.tile_pool(name="consts", bufs=1))
        big = ctx.enter_context(tc.tile_pool(name="big", bufs=1))
        ppool = ctx.enter_context(tc.tile_pool(name="pn", bufs=2))
        opool = ctx.enter_context(tc.tile_pool(name="small", bufs=2))
        ps_s = ctx.enter_context(tc.tile_pool(name="ps_s", bufs=2, space="PSUM"))
        ps_o = ctx.enter_context(tc.tile_pool(name="ps_o", bufs=2, space="PSUM"))

        # ---- constants / weights to SBUF ----
        wq_sb = consts.tile([128, 128], BF16, tag="wq")
        wk0_sb = consts.tile([128, 128], BF16, tag="wk0")
        wk1_sb = consts.tile([128, 128], BF16, tag="wk1")
        wp0_sb = consts.tile([128, 128], BF16, tag="wp0")
        wp1_sb = consts.tile([128, 128], BF16, tag="wp1")
        bq_sb = consts.tile([128, 1], F32, tag="bq")
        nc.sync.dma_start(wq_sb[:], wq_t[:])
        nc.sync.dma_start(wk0_sb[:], wk_t0[:])
        nc.sync.dma_start(wk1_sb[:], wk_t1[:])
        nc.sync.dma_start(wp0_sb[:], wp_t0[:])
        nc.sync.dma_start(wp1_sb[:], wp_t1[:])
        nc.sync.dma_start(bq_sb[:], bq_d[:])

        cape_sb = big.tile([128, N], BF16, tag="cape")
        era5a_sb = big.tile([128, N], BF16, tag="era5a")
        era5b_sb = big.tile([128, N], BF16, tag="era5b")
        nc.sync.dma_start(cape_sb[:], cape[:])
        nc.sync.dma_start(era5a_sb[:], era5a[:])
        nc.sync.dma_start(era5b_sb[:], era5b[:])

        q_sb = big.tile([128, N], BF16, tag="q")
        k_sb = big.tile([128, N], BF16, tag="k")
        vT_sb = big.tile([128, NKC * VSTride], BF16, tag="vT")

        # ---- projections ----
        # k = Wk @ era5                [D, N]   (first: the S matmuls need all of k)
        for j in range(8):
            sl = slice(j * 512, (j + 1) * 512)
            pk = ps_s.tile([128, 512], F32, tag="s")
            nc.tensor.matmul(pk[:], wk0_sb[:], era5a_sb[:, sl], start=True, stop=False)
            nc.tensor.matmul(pk[:], wk1_sb[:], era5b_sb[:, sl], start=False, stop=True)
            nc.scalar.activation(k_sb[:, sl], pk[:], AFT.Copy)
        # q = (s Wq) @ cape + s bq     [D, N]
        for j in range(8):
            sl = slice(j * 512, (j + 1) * 512)
            pq = ps_s.tile([128, 512], F32, tag="s")
            nc.tensor.matmul(pq[:], wq_sb[:], cape_sb[:, sl])
            nc.vector.tensor_scalar_add(q_sb[:, sl], pq[:], bq_sb[:])
        # v'T chunks: v'T[kk, d] = era5^T @ (Wo Wv)^T, chunk kk of 128
        vT_view = vT_sb.rearrange("p (c x) -> p c x", x=VSTride)
        for c4 in range(NKC // 4):
            pv = ps_s.tile([128, 512], F32, tag="s")
            for i in range(4):
                c = c4 * 4 + i
                ksl = slice(c * 128, (c + 1) * 128)
                osl = slice(i * 128, (i + 1) * 128)
                nc.tensor.matmul(pv[:, osl], era5a_sb[:, ksl], wp0_sb[:],
                                 start=True, stop=False)
                nc.tensor.matmul(pv[:, osl], era5b_sb[:, ksl], wp1_sb[:],
                                 start=False, stop=True)
            nc.vector.tensor_copy(
                vT_view[:, c4 * 4:(c4 + 1) * 4, 0:128],
                pv[:].rearrange("p (c x) -> p c x", x=128))
        # ones column (softmax denominator) per v'T chunk
        nc.gpsimd.memset(vT_view[:, :, 128:129], 1.0)

        # ---- main attention loop over query superblocks of 512 ----
        # Software-pipelined: superblock s runs S^T+exp while PE also runs the
        # value matmuls (VP) of superblock s-1 from its staged P buffer.
        p_bufs = {}       # s -> [128, 8192] bf16 staged exp(S^T)
        o_tiles = {}      # (s, j) -> [128, 129] psum accumulator

        def emit_vp_group(s, j, c_lo, c_hi):
            """VP matmuls for superblock s, query sub-block j, chunks [c_lo, c_hi)."""
            o_t = o_tiles[(s, j)]
            p_b = p_bufs[s]
            for c in range(c_lo, c_hi):
                lhs = p_b[:, c * 512 + j * 128: c * 512 + j * 128 + 128]
                nc.tensor.matmul(o_t[:], lhs,
                                 vT_sb[:, c * VSTride:c * VSTride + 129],
                                 start=(c == 0), stop=(c == NKC - 1))

        def emit_post(s, j):
            o_t = o_tiles.pop((s, j))
            recip_t = opool.tile([128, 1], F32, tag="recip")
            nc.vector.reciprocal(recip_t[:], o_t[:, 128:129])
            nrm_t = opool.tile([128, 128], F32, tag="nrm")
            nc.vector.tensor_scalar_mul(nrm_t[:], o_t[:, 0:128], recip_t[:])
            row = s * 512 + j * 128
            nc.sync.dma_start(out_d[row:row + 128, :], nrm_t[:])

        # VP work for superblock s-1 is spread over the 11 exp-group slots of
        # superblock s, j-major so at most 2 o_tiles are live.
        vp_sched = []     # per group-slot: list of (j, c_lo, c_hi)
        per_slot = (4 * NKC) // len(GROUPS) + 1   # ~12 chunk-MMs per slot
        flat = [(j, c) for j in range(4) for c in range(NKC)]
        for gi in range(len(GROUPS)):
            chunk = flat[gi * per_slot:(gi + 1) * per_slot]
            sched = []
            for (j, c) in chunk:
                if sched and sched[-1][0] == j and sched[-1][2] == c:
                    sched[-1] = (j, sched[-1][1], c + 1)
                else:
                    sched.append((j, c, c + 1))
            vp_sched.append(sched)

        for s in range(NSB):
            qsl = slice(s * 512, (s + 1) * 512)
            p_b = ppool.tile([128, NKC * 512], BF16, tag="p")
            p_bufs[s] = p_b
            c0 = 0
            for gi, G in enumerate(GROUPS):
                s_tile = ps_s.tile([128, G * 512], F32, tag="s")
                for i in range(G):
                    c = c0 + i
                    nc.tensor.matmul(s_tile[:, i * 512:(i + 1) * 512],
                                     k_sb[:, c * 128:(c + 1) * 128],
                                     q_sb[:, qsl])
                nc.scalar.activation(
                    p_b[:, c0 * 512:(c0 + G) * 512], s_tile[:], AFT.Exp)
                c0 += G
                # interleave previous superblock's VP + posts
                if s > 0:
                    for (j, c_lo, c_hi) in vp_sched[gi]:
                        if c_lo == 0:
                            o_tiles[(s - 1, j)] = ps_o.tile([128, 129], F32, tag="o", name=f"o_{s-1}_{j}")
                        emit_vp_group(s - 1, j, c_lo, c_hi)
                        if c_hi == NKC:
                            emit_post(s - 1, j)
            if s > 0:
                p_bufs.pop(s - 1)

        # pipeline tail: VP + post of the last superblock
        s = NSB - 1
        for j in range(4):
            o_tiles[(s, j)] = ps_o.tile([128, 129], F32, tag="o", name=f"o_{s}_{j}")
            emit_vp_group(s, j, 0, NKC)
            emit_post(s, j)

    nc.compile()
    return nc


def _get_program():
    if "nc" not in _CACHE:
        _CACHE["nc"] = build_program()
    return _CACHE["nc"]


def kernel(cape_features, era5_features, Wq, bq, Wk, bk, Wv, bv, Wo, bo):
    global LAST_RESULTS
    bf = ml_dtypes.bfloat16
    cape = np.asarray(cape_features, np.float32)
    era5 = np.asarray(era5_features, np.float32)
    Wq = np.asarray(Wq, np.float32)
    bq = np.asarray(bq, np.float32)
    Wk = np.asarray(Wk, np.float32)
    Wv = np.asarray(Wv, np.float32)
    bv = np.asarray(bv, np.float32)
    Wo = np.asarray(Wo, np.float32)
    bo = np.asarray(bo, np.float32)

    B = cape.shape[0]
    scale = np.float32(Wq.shape[0] ** -0.5)

    wq_t = np.ascontiguousarray((Wq * scale).T).astype(bf)       # [Cc, D]
    wk_t = np.ascontiguousarray(Wk.T)                            # [Ce, D]
    Wp = Wo @ Wv                                                 # [Cc, Ce]
    wp_t = np.ascontiguousarray(Wp.T)                            # [Ce, Cc]
    bq_e = np.ascontiguousarray((bq * scale).reshape(128, 1), dtype=np.float32)
    bp_e = (Wo @ bv + bo).astype(np.float32)          # added host-side

    common = {
        "wq_t": wq_t,
        "wk_t0": wk_t[:128].astype(bf), "wk_t1": wk_t[128:].astype(bf),
        "wp_t0": wp_t[:128].astype(bf), "wp_t1": wp_t[128:].astype(bf),
        "bq": bq_e,
    }
    in_maps = []
    for s in range(B):
        e = era5[s].reshape(256, N)
        in_maps.append(dict(common,
                            cape=cape[s].reshape(128, N).astype(bf),
                            era5a=e[:128].astype(bf),
                            era5b=e[128:].astype(bf)))

    nc = _get_program()
    res = run_bass_kernel_spmd(
        nc, in_maps, core_ids=list(range(NCORES)),
        trace=bool(int(os.environ.get("KBENCH_TRACE", "0"))),
    )
    LAST_RESULTS = res
    out = np.stack([
        (res.results[s]["out"].T + bp_e[:, None]).reshape(128, 64, 64)
        for s in range(B)
    ])
    return np.ascontiguousarray(out, dtype=np.float32)


# revision 13
# speedup vs baseline: 1.2126x; 1.0173x over previous
"""Cross-modal attention kernel for Trainium2 (Bass/Tile), data-parallel over
batch across 8 NeuronCores.

Math (per batch sample, N = 64*64 = 4096, D = 128):
    q = (s*Wq) @ cape + s*bq          [D, N]   (s = D**-0.5 folded into Wq,bq)
    k = Wk @ era5                     [D, N]   (bk dropped: constant along the
                                               softmax axis, cancels)
    S^T = k^T q                       [N, N]   computed in [128kk x 128qq] tiles
    P = exp(S^T)                      softmax numerator, kk on partitions
    U = (Wo@Wv @ era5) @ P            [128, N] Wo folded into V; softmax
                                               denominator = ones-column of the
                                               rhs -> column 128 of the output
    out = U[:, :128]/denom + (Wo@bv + bo)

Normalization is deferred past the value/output projections (both linear per
query column), so no per-element multiply over the NxN attention matrix is
ever needed; the denominator rides along as a 129th matmul output column.
"""

import os
import numpy as np
from contextlib import ExitStack

import concourse.bass as bass
import concourse.bacc as bacc
import concourse.mybir as mybir
import concourse.tile as tile
from concourse.bass_utils import run_bass_kernel_spmd
import ml_dtypes

AFT = mybir.ActivationFunctionType
BF16 = mybir.dt.bfloat16
F32 = mybir.dt.float32

N = 4096          # h*w
D = 128           # attn dim == cape channels
NCORES = 8
NKC = N // 128    # 32 kk chunks of 128
NSB = N // 512    # 8 query superblocks of 512
GROUPS = (3, 3, 3, 3, 3, 3, 3, 3, 3, 3, 2)   # kk chunks per exp group
VSTride = 136     # free-dim stride of one v'T chunk in SBUF (128 data + ones + pad)

_CACHE = {}
LAST_RESULTS = None


def build_program():
    nc = bacc.Bacc("TRN2", debug=False, target_bir_lowering=False)

    cape = nc.dram_tensor("cape", [128, N], BF16, kind="ExternalInput")
    era5a = nc.dram_tensor("era5a", [128, N], BF16, kind="ExternalInput")
    era5b = nc.dram_tensor("era5b", [128, N], BF16, kind="ExternalInput")
    wq_t = nc.dram_tensor("wq_t", [128, 128], BF16, kind="ExternalInput")
    wk_t0 = nc.dram_tensor("wk_t0", [128, 128], BF16, kind="ExternalInput")
    wk_t1 = nc.dram_tensor("wk_t1", [128, 128], BF16, kind="ExternalInput")
    wp_t0 = nc.dram_tensor("wp_t0", [128, 128], BF16, kind="ExternalInput")
    wp_t1 = nc.dram_tensor("wp_t1", [128, 128], BF16, kind="ExternalInput")
    bq_d = nc.dram_tensor("bq", [128, 1], F32, kind="ExternalInput")
    # output is stored TRANSPOSED: [N, 128] = (out + bias)^T without bias;
    # host adds the (folded) bias and transposes back.
    out_d = nc.dram_tensor("out", [N, 128], F32, kind="ExternalOutput")

    with tile.TileContext(nc) as tc, ExitStack() as ctx:
        consts = ctx.enter_context(tc.tile_pool(name="consts", bufs=1))
        big = ctx.enter_context(tc.tile_pool(name="big", bufs=1))
        ppool = ctx.enter_context(tc.tile_pool(name="pn", bufs=2))
        opool = ctx.enter_context(tc.tile_pool(name="small", bufs=2))
        ps_s = ctx.enter_context(tc.tile_pool(name="ps_s", bufs=2, space="PSUM"))
        ps_o = ctx.enter_context(tc.tile_pool(name="ps_o", bufs=2, space="PSUM"))

        # ---- constants / weights to SBUF ----
        wq_sb = consts.tile([128, 128], BF16, tag="wq")
        wk0_sb = consts.tile([128, 128], BF16, tag="wk0")
        wk1_sb = consts.tile([128, 128], BF16, tag="wk1")
        wp0_sb = consts.tile([128, 128], BF16, tag="wp0")
        wp1_sb = consts.tile([128, 128], BF16, tag="wp1")
        bq_sb = consts.tile([128, 1], F32, tag="bq")
        nc.sync.dma_start(wq_sb[:], wq_t[:])
        nc.sync.dma_start(wk0_sb[:], wk_t0[:])
        nc.sync.dma_start(wk1_sb[:], wk_t1[:])
        nc.sync.dma_start(wp0_sb[:], wp_t0[:])
        nc.sync.dma_start(wp1_sb[:], wp_t1[:])
        nc.sync.dma_start(bq_sb[:], bq_d[:])

        cape_sb = big.tile([128, N], BF16, tag="cape")
        era5a_sb = big.tile([128, N], BF16, tag="era5a")
        era5b_sb = big.tile([128, N], BF16, tag="era5b")
        # chunked so the loads spread across DMA queues (era5 first: k needs it)
        for j in range(4):
            sl = slice(j * 1024, (j + 1) * 1024)
            nc.sync.dma_start(era5a_sb[:, sl], era5a[:, sl])
            nc.sync.dma_start(era5b_sb[:, sl], era5b[:, sl])
        for j in range(4):
            sl = slice(j * 1024, (j + 1) * 1024)
            nc.sync.dma_start(cape_sb[:, sl], cape[:, sl])

        q_sb = big.tile([128, N], BF16, tag="q")
        k_sb = big.tile([128, N], BF16, tag="k")
        vT_sb = big.tile([128, NKC * VSTride], BF16, tag="vT")

        # ---- projections ----
        # k = Wk @ era5                [D, N]   (first: the S matmuls need all of k)
        for j in range(8):
            sl = slice(j * 512, (j + 1) * 512)
            pk = ps_s.tile([128, 512], F32, tag="s")
            nc.tensor.matmul(pk[:], wk0_sb[:], era5a_sb[:, sl], start=True, stop=False)
            nc.tensor.matmul(pk[:], wk1_sb[:], era5b_sb[:, sl], start=False, stop=True)
            nc.scalar.activation(k_sb[:, sl], pk[:], AFT.Copy)
        # q = (s Wq) @ cape + s bq     [D, N]
        for j in range(8):
            sl = slice(j * 512, (j + 1) * 512)
            pq = ps_s.tile([128, 512], F32, tag="s")
            nc.tensor.matmul(pq[:], wq_sb[:], cape_sb[:, sl])
            nc.vector.tensor_scalar_add(q_sb[:, sl], pq[:], bq_sb[:])
        # v'T chunks: v'T[kk, d] = era5^T @ (Wo Wv)^T, chunk kk of 128
        vT_view = vT_sb.rearrange("p (c x) -> p c x", x=VSTride)
        for c4 in range(NKC // 4):
            pv = ps_s.tile([128, 512], F32, tag="s")
            for i in range(4):
                c = c4 * 4 + i
                ksl = slice(c * 128, (c + 1) * 128)
                osl = slice(i * 128, (i + 1) * 128)
                nc.tensor.matmul(pv[:, osl], era5a_sb[:, ksl], wp0_sb[:],
                                 start=True, stop=False)
                nc.tensor.matmul(pv[:, osl], era5b_sb[:, ksl], wp1_sb[:],
                                 start=False, stop=True)
            nc.vector.tensor_copy(
                vT_view[:, c4 * 4:(c4 + 1) * 4, 0:128],
                pv[:].rearrange("p (c x) -> p c x", x=128))
        # ones column (softmax denominator) per v'T chunk
        nc.gpsimd.memset(vT_view[:, :, 128:129], 1.0)

        # ---- main attention loop over query superblocks of 512 ----
        # Software-pipelined: superblock s runs S^T+exp while PE also runs the
        # value matmuls (VP) of superblock s-1 from its staged P buffer.
        p_bufs = {}       # s -> [128, 8192] bf16 staged exp(S^T)
        o_tiles = {}      # (s, j) -> [128, 129] psum accumulator

        def emit_vp_group(s, j, c_lo, c_hi):
            """VP matmuls for superblock s, query sub-block j, chunks [c_lo, c_hi)."""
            o_t = o_tiles[(s, j)]
            p_b = p_bufs[s]
            for c in range(c_lo, c_hi):
                lhs = p_b[:, c * 512 + j * 128: c * 512 + j * 128 + 128]
                nc.tensor.matmul(o_t[:], lhs,
                                 vT_sb[:, c * VSTride:c * VSTride + 129],
                                 start=(c == 0), stop=(c == NKC - 1))

        def emit_post(s, j):
            o_t = o_tiles.pop((s, j))
            recip_t = opool.tile([128, 1], F32, tag="recip")
            nc.vector.reciprocal(recip_t[:], o_t[:, 128:129])
            nrm_t = opool.tile([128, 128], F32, tag="nrm")
            nc.vector.tensor_scalar_mul(nrm_t[:], o_t[:, 0:128], recip_t[:])
            row = s * 512 + j * 128
            nc.sync.dma_start(out_d[row:row + 128, :], nrm_t[:])

        # VP work for superblock s-1 is spread over the 11 exp-group slots of
        # superblock s, j-major so at most 2 o_tiles are live.
        vp_sched = []     # per group-slot: list of (j, c_lo, c_hi)
        per_slot = (4 * NKC) // len(GROUPS) + 1   # ~12 chunk-MMs per slot
        flat = [(j, c) for j in range(4) for c in range(NKC)]
        for gi in range(len(GROUPS)):
            chunk = flat[gi * per_slot:(gi + 1) * per_slot]
            sched = []
            for (j, c) in chunk:
                if sched and sched[-1][0] == j and sched[-1][2] == c:
                    sched[-1] = (j, sched[-1][1], c + 1)
                else:
                    sched.append((j, c, c + 1))
            vp_sched.append(sched)

        for s in range(NSB):
            qsl = slice(s * 512, (s + 1) * 512)
            p_b = ppool.tile([128, NKC * 512], BF16, tag="p")
            p_bufs[s] = p_b
            c0 = 0
            for gi, G in enumerate(GROUPS):
                s_tile = ps_s.tile([128, G * 512], F32, tag="s")
                for i in range(G):
                    c = c0 + i
                    nc.tensor.matmul(s_tile[:, i * 512:(i + 1) * 512],
                                     k_sb[:, c * 128:(c + 1) * 128],
                                     q_sb[:, qsl])
                nc.scalar.activation(
                    p_b[:, c0 * 512:(c0 + G) * 512], s_tile[:], AFT.Exp)
                c0 += G
                # interleave previous superblock's VP + posts
                if s > 0:
                    for (j, c_lo, c_hi) in vp_sched[gi]:
                        if c_lo == 0:
                            o_tiles[(s - 1, j)] = ps_o.tile([128, 129], F32, tag="o", name=f"o_{s-1}_{j}")
                        emit_vp_group(s - 1, j, c_lo, c_hi)
                        if c_hi == NKC:
                            emit_post(s - 1, j)
            if s > 0:
                p_bufs.pop(s - 1)

        # pipeline tail: VP + post of the last superblock
        s = NSB - 1
        for j in range(4):
            o_tiles[(s, j)] = ps_o.tile([128, 129], F32, tag="o", name=f"o_{s}_{j}")
            emit_vp_group(s, j, 0, NKC)
            emit_post(s, j)

    nc.compile()
    return nc


def _get_program():
    if "nc" not in _CACHE:
        _CACHE["nc"] = build_program()
    return _CACHE["nc"]


def kernel(cape_features, era5_features, Wq, bq, Wk, bk, Wv, bv, Wo, bo):
    global LAST_RESULTS
    bf = ml_dtypes.bfloat16
    cape = np.asarray(cape_features, np.float32)
    era5 = np.asarray(era5_features, np.float32)
    Wq = np.asarray(Wq, np.float32)
    bq = np.asarray(bq, np.float32)
    Wk = np.asarray(Wk, np.float32)
    Wv = np.asarray(Wv, np.float32)
    bv = np.asarray(bv, np.float32)
    Wo = np.asarray(Wo, np.float32)
    bo = np.asarray(bo, np.float32)

    B = cape.shape[0]
    scale = np.float32(Wq.shape[0] ** -0.5)

    wq_t = np.ascontiguousarray((Wq * scale).T).astype(bf)       # [Cc, D]
    wk_t = np.ascontiguousarray(Wk.T)                            # [Ce, D]
    Wp = Wo @ Wv                                                 # [Cc, Ce]
    wp_t = np.ascontiguousarray(Wp.T)                            # [Ce, Cc]
    bq_e = np.ascontiguousarray((bq * scale).reshape(128, 1), dtype=np.float32)
    bp_e = (Wo @ bv + bo).astype(np.float32)          # added host-side

    common = {
        "wq_t": wq_t,
        "wk_t0": wk_t[:128].astype(bf), "wk_t1": wk_t[128:].astype(bf),
        "wp_t0": wp_t[:128].astype(bf), "wp_t1": wp_t[128:].astype(bf),
        "bq": bq_e,
    }
    in_maps = []
    for s in range(B):
        e = era5[s].reshape(256, N)
        in_maps.append(dict(common,
                            cape=cape[s].reshape(128, N).astype(bf),
                            era5a=e[:128].astype(bf),
                            era5b=e[128:].astype(bf)))

    nc = _get_program()
    res = run_bass_kernel_spmd(
        nc, in_maps, core_ids=list(range(NCORES)),
        trace=bool(int(os.environ.get("KBENCH_TRACE", "0"))),
    )
    LAST_RESULTS = res
    out = np.stack([
        (res.results[s]["out"].T + bp_e[:, None]).reshape(128, 64, 64)
        for s in range(B)
    ])
    return np.ascontiguousarray(out, dtype=np.float32)


# revision 15
# speedup vs baseline: 1.2157x; 1.0026x over previous
"""Cross-modal attention kernel for Trainium2 (Bass/Tile), data-parallel over
batch across 8 NeuronCores.

Math (per batch sample, N = 64*64 = 4096, D = 128):
    q = (s*Wq) @ cape + s*bq          [D, N]   (s = D**-0.5 folded into Wq,bq)
    k = Wk @ era5                     [D, N]   (bk dropped: constant along the
                                               softmax axis, cancels)
    S^T = k^T q                       [N, N]   computed in [128kk x 128qq] tiles
    P = exp(S^T)                      softmax numerator, kk on partitions
    U = (Wo@Wv @ era5) @ P            [128, N] Wo folded into V; softmax
                                               denominator = ones-column of the
                                               rhs -> column 128 of the output
    out = U[:, :128]/denom + (Wo@bv + bo)

Normalization is deferred past the value/output projections (both linear per
query column), so no per-element multiply over the NxN attention matrix is
ever needed; the denominator rides along as a 129th matmul output column.
"""

import os
import numpy as np
from contextlib import ExitStack

import concourse.bass as bass
import concourse.bacc as bacc
import concourse.mybir as mybir
import concourse.tile as tile
from concourse.bass_utils import run_bass_kernel_spmd
import ml_dtypes

AFT = mybir.ActivationFunctionType
BF16 = mybir.dt.bfloat16
F32 = mybir.dt.float32

N = 4096          # h*w
D = 128           # attn dim == cape channels
NCORES = 8
NKC = N // 128    # 32 kk chunks of 128
NSB = N // 512    # 8 query superblocks of 512
GROUPS = (3, 3, 3, 3, 3, 3, 3, 3, 3, 3, 2)   # kk chunks per exp group
VSTride = 136     # free-dim stride of one v'T chunk in SBUF (128 data + ones + pad)

_CACHE = {}
LAST_RESULTS = None


def build_program():
    nc = bacc.Bacc("TRN2", debug=False, target_bir_lowering=False)

    cape = nc.dram_tensor("cape", [128, N], BF16, kind="ExternalInput")
    era5a = nc.dram_tensor("era5a", [128, N], BF16, kind="ExternalInput")
    era5b = nc.dram_tensor("era5b", [128, N], BF16, kind="ExternalInput")
    wq_t = nc.dram_tensor("wq_t", [128, 128], BF16, kind="ExternalInput")
    wk_t0 = nc.dram_tensor("wk_t0", [128, 128], BF16, kind="ExternalInput")
    wk_t1 = nc.dram_tensor("wk_t1", [128, 128], BF16, kind="ExternalInput")
    wp_t0 = nc.dram_tensor("wp_t0", [128, 128], BF16, kind="ExternalInput")
    wp_t1 = nc.dram_tensor("wp_t1", [128, 128], BF16, kind="ExternalInput")
    bq_d = nc.dram_tensor("bq", [128, 1], F32, kind="ExternalInput")
    # output is stored TRANSPOSED: [N, 128] = (out + bias)^T without bias;
    # host adds the (folded) bias and transposes back.
    out_d = nc.dram_tensor("out", [N, 128], F32, kind="ExternalOutput")

    with tile.TileContext(nc) as tc, ExitStack() as ctx:
        consts = ctx.enter_context(tc.tile_pool(name="consts", bufs=1))
        big = ctx.enter_context(tc.tile_pool(name="big", bufs=1))
        ppool = ctx.enter_context(tc.tile_pool(name="pn", bufs=2))
        opool = ctx.enter_context(tc.tile_pool(name="small", bufs=2))
        ps_s = ctx.enter_context(tc.tile_pool(name="ps_s", bufs=2, space="PSUM"))
        ps_o = ctx.enter_context(tc.tile_pool(name="ps_o", bufs=2, space="PSUM"))

        # ---- constants / weights to SBUF ----
        wq_sb = consts.tile([128, 128], BF16, tag="wq")
        wk0_sb = consts.tile([128, 128], BF16, tag="wk0")
        wk1_sb = consts.tile([128, 128], BF16, tag="wk1")
        wp0_sb = consts.tile([128, 128], BF16, tag="wp0")
        wp1_sb = consts.tile([128, 128], BF16, tag="wp1")
        bq_sb = consts.tile([128, 1], F32, tag="bq")
        nc.sync.dma_start(wq_sb[:], wq_t[:])
        nc.sync.dma_start(wk0_sb[:], wk_t0[:])
        nc.sync.dma_start(wk1_sb[:], wk_t1[:])
        nc.sync.dma_start(wp0_sb[:], wp_t0[:])
        nc.sync.dma_start(wp1_sb[:], wp_t1[:])
        nc.sync.dma_start(bq_sb[:], bq_d[:])

        cape_sb = big.tile([128, N], BF16, tag="cape")
        era5a_sb = big.tile([128, N], BF16, tag="era5a")
        era5b_sb = big.tile([128, N], BF16, tag="era5b")
        # chunked so the loads spread across DMA queues (era5 first: k needs it)
        for j in range(4):
            sl = slice(j * 1024, (j + 1) * 1024)
            nc.sync.dma_start(era5a_sb[:, sl], era5a[:, sl])
            nc.sync.dma_start(era5b_sb[:, sl], era5b[:, sl])
        for j in range(4):
            sl = slice(j * 1024, (j + 1) * 1024)
            nc.sync.dma_start(cape_sb[:, sl], cape[:, sl])

        q_sb = big.tile([128, N], BF16, tag="q")
        k_sb = big.tile([128, N], BF16, tag="k")
        vT_sb = big.tile([128, NKC * VSTride], BF16, tag="vT")

        # ---- projections ----
        # k = Wk @ era5                [D, N]   (first: the S matmuls need all
        # of k; PSUM->SBUF copies split across ScalarE/VectorE to halve the wall)
        for j in range(8):
            sl = slice(j * 512, (j + 1) * 512)
            pk = ps_s.tile([128, 512], F32, tag="s")
            nc.tensor.matmul(pk[:], wk0_sb[:], era5a_sb[:, sl], start=True, stop=False)
            nc.tensor.matmul(pk[:], wk1_sb[:], era5b_sb[:, sl], start=False, stop=True)
            if j % 2 == 0:
                nc.scalar.activation(k_sb[:, sl], pk[:], AFT.Copy)
            else:
                nc.vector.tensor_copy(k_sb[:, sl], pk[:])
        # q = (s Wq) @ cape + s bq     [D, N]   (block 0 first: it gates exp #1)
        for j in range(8):
            sl = slice(j * 512, (j + 1) * 512)
            pq = ps_s.tile([128, 512], F32, tag="s")
            nc.tensor.matmul(pq[:], wq_sb[:], cape_sb[:, sl])
            nc.vector.tensor_scalar_add(q_sb[:, sl], pq[:], bq_sb[:])
        # v'T chunks (v'T[kk, d] = era5^T @ (Wo Wv)^T) are generated INSIDE
        # superblock 0's group slots, where the PE would otherwise idle.
        vT_view = vT_sb.rearrange("p (c x) -> p c x", x=VSTride)

        def emit_vt_group(c4):
            pv = ps_s.tile([128, 512], F32, tag="s", name=f"pv_{c4}")
            for i in range(4):
                c = c4 * 4 + i
                ksl = slice(c * 128, (c + 1) * 128)
                osl = slice(i * 128, (i + 1) * 128)
                nc.tensor.matmul(pv[:, osl], era5a_sb[:, ksl], wp0_sb[:],
                                 start=True, stop=False)
                nc.tensor.matmul(pv[:, osl], era5b_sb[:, ksl], wp1_sb[:],
                                 start=False, stop=True)
            nc.vector.tensor_copy(
                vT_view[:, c4 * 4:(c4 + 1) * 4, 0:128],
                pv[:].rearrange("p (c x) -> p c x", x=128))
        # ones column (softmax denominator) per v'T chunk
        nc.gpsimd.memset(vT_view[:, :, 128:129], 1.0)

        # ---- main attention loop over query superblocks of 512 ----
        # Software-pipelined: superblock s runs S^T+exp while PE also runs the
        # value matmuls (VP) of superblock s-1 from its staged P buffer.
        p_bufs = {}       # s -> [128, 8192] bf16 staged exp(S^T)
        o_tiles = {}      # (s, j) -> [128, 129] psum accumulator

        def emit_vp_group(s, j, c_lo, c_hi):
            """VP matmuls for superblock s, query sub-block j, chunks [c_lo, c_hi)."""
            o_t = o_tiles[(s, j)]
            p_b = p_bufs[s]
            for c in range(c_lo, c_hi):
                lhs = p_b[:, c * 512 + j * 128: c * 512 + j * 128 + 128]
                nc.tensor.matmul(o_t[:], lhs,
                                 vT_sb[:, c * VSTride:c * VSTride + 129],
                                 start=(c == 0), stop=(c == NKC - 1))

        def emit_post(s, j):
            o_t = o_tiles.pop((s, j))
            recip_t = opool.tile([128, 1], F32, tag="recip")
            nc.vector.reciprocal(recip_t[:], o_t[:, 128:129])
            nrm_t = opool.tile([128, 128], F32, tag="nrm")
            nc.vector.tensor_scalar_mul(nrm_t[:], o_t[:, 0:128], recip_t[:])
            row = s * 512 + j * 128
            nc.sync.dma_start(out_d[row:row + 128, :], nrm_t[:])

        # VP work for superblock s-1 is spread over the 11 exp-group slots of
        # superblock s, j-major so at most 2 o_tiles are live.
        vp_sched = []     # per group-slot: list of (j, c_lo, c_hi)
        per_slot = (4 * NKC) // len(GROUPS) + 1   # ~12 chunk-MMs per slot
        flat = [(j, c) for j in range(4) for c in range(NKC)]
        for gi in range(len(GROUPS)):
            chunk = flat[gi * per_slot:(gi + 1) * per_slot]
            sched = []
            for (j, c) in chunk:
                if sched and sched[-1][0] == j and sched[-1][2] == c:
                    sched[-1] = (j, sched[-1][1], c + 1)
                else:
                    sched.append((j, c, c + 1))
            vp_sched.append(sched)

        for s in range(NSB):
            qsl = slice(s * 512, (s + 1) * 512)
            p_b = ppool.tile([128, NKC * 512], BF16, tag="p")
            p_bufs[s] = p_b
            c0 = 0
            for gi, G in enumerate(GROUPS):
                s_tile = ps_s.tile([128, G * 512], F32, tag="s")
                for i in range(G):
                    c = c0 + i
                    nc.tensor.matmul(s_tile[:, i * 512:(i + 1) * 512],
                                     k_sb[:, c * 128:(c + 1) * 128],
                                     q_sb[:, qsl])
                nc.scalar.activation(
                    p_b[:, c0 * 512:(c0 + G) * 512], s_tile[:], AFT.Exp)
                c0 += G
                # interleave previous superblock's VP + posts (superblock 0
                # interleaves the v'T generation instead)
                if s > 0:
                    for (j, c_lo, c_hi) in vp_sched[gi]:
                        if c_lo == 0:
                            o_tiles[(s - 1, j)] = ps_o.tile([128, 129], F32, tag="o", name=f"o_{s-1}_{j}")
                        emit_vp_group(s - 1, j, c_lo, c_hi)
                        if c_hi == NKC:
                            emit_post(s - 1, j)
                elif gi < NKC // 4:
                    emit_vt_group(gi)
            if s > 0:
                p_bufs.pop(s - 1)

        # pipeline tail: VP + post of the last superblock
        s = NSB - 1
        for j in range(4):
            o_tiles[(s, j)] = ps_o.tile([128, 129], F32, tag="o", name=f"o_{s}_{j}")
            emit_vp_group(s, j, 0, NKC)
            emit_post(s, j)

    nc.compile()
    return nc


def _get_program():
    if "nc" not in _CACHE:
        _CACHE["nc"] = build_program()
    return _CACHE["nc"]


def kernel(cape_features, era5_features, Wq, bq, Wk, bk, Wv, bv, Wo, bo):
    global LAST_RESULTS
    bf = ml_dtypes.bfloat16
    cape = np.asarray(cape_features, np.float32)
    era5 = np.asarray(era5_features, np.float32)
    Wq = np.asarray(Wq, np.float32)
    bq = np.asarray(bq, np.float32)
    Wk = np.asarray(Wk, np.float32)
    Wv = np.asarray(Wv, np.float32)
    bv = np.asarray(bv, np.float32)
    Wo = np.asarray(Wo, np.float32)
    bo = np.asarray(bo, np.float32)

    B = cape.shape[0]
    scale = np.float32(Wq.shape[0] ** -0.5)

    wq_t = np.ascontiguousarray((Wq * scale).T).astype(bf)       # [Cc, D]
    wk_t = np.ascontiguousarray(Wk.T)                            # [Ce, D]
    Wp = Wo @ Wv                                                 # [Cc, Ce]
    wp_t = np.ascontiguousarray(Wp.T)                            # [Ce, Cc]
    bq_e = np.ascontiguousarray((bq * scale).reshape(128, 1), dtype=np.float32)
    bp_e = (Wo @ bv + bo).astype(np.float32)          # added host-side

    common = {
        "wq_t": wq_t,
        "wk_t0": wk_t[:128].astype(bf), "wk_t1": wk_t[128:].astype(bf),
        "wp_t0": wp_t[:128].astype(bf), "wp_t1": wp_t[128:].astype(bf),
        "bq": bq_e,
    }
    in_maps = []
    for s in range(B):
        e = era5[s].reshape(256, N)
        in_maps.append(dict(common,
                            cape=cape[s].reshape(128, N).astype(bf),
                            era5a=e[:128].astype(bf),
                            era5b=e[128:].astype(bf)))

    nc = _get_program()
    res = run_bass_kernel_spmd(
        nc, in_maps, core_ids=list(range(NCORES)),
        trace=bool(int(os.environ.get("KBENCH_TRACE", "0"))),
    )
    LAST_RESULTS = res
    out = np.stack([
        (res.results[s]["out"].T + bp_e[:, None]).reshape(128, 64, 64)
        for s in range(B)
    ])
    return np.ascontiguousarray(out, dtype=np.float32)


# revision 18
# speedup vs baseline: 1.2926x; 1.0632x over previous
"""Cross-modal attention kernel for Trainium2 (Bass/Tile), data-parallel over
batch across 8 NeuronCores.

Math (per batch sample, N = 64*64 = 4096, D = 128):
    q = (s*Wq) @ cape + s*bq          [D, N]   (s = D**-0.5 folded into Wq,bq)
    k = Wk @ era5                     [D, N]   (bk dropped: constant along the
                                               softmax axis, cancels)
    S^T = k^T q                       [N, N]   computed in [128kk x 128qq] tiles
    P = exp(S^T)                      softmax numerator, kk on partitions
    U = (Wo@Wv @ era5) @ P            [128, N] Wo folded into V; softmax
                                               denominator = ones-column of the
                                               rhs -> column 128 of the output
    out = U[:, :128]/denom + (Wo@bv + bo)

Normalization is deferred past the value/output projections (both linear per
query column), so no per-element multiply over the NxN attention matrix is
ever needed; the denominator rides along as a 129th matmul output column.
"""

import os
import numpy as np
from contextlib import ExitStack

import concourse.bass as bass
import concourse.bacc as bacc
import concourse.mybir as mybir
import concourse.tile as tile
from concourse.bass_utils import run_bass_kernel_spmd
import ml_dtypes

AFT = mybir.ActivationFunctionType
BF16 = mybir.dt.bfloat16
F32 = mybir.dt.float32

N = 4096          # h*w
D = 128           # attn dim == cape channels
NCORES = 8
NKC = N // 128    # 32 kk chunks of 128
NSB = N // 512    # 8 query superblocks of 512
GROUPS = (3, 3, 3, 3, 3, 3, 3, 3, 3, 3, 2)   # kk chunks per exp group
VSTride = 136     # free-dim stride of one v'T chunk in SBUF (128 data + ones + pad)

_CACHE = {}
LAST_RESULTS = None


def build_program():
    nc = bacc.Bacc("TRN2", debug=False, target_bir_lowering=False)

    cape = nc.dram_tensor("cape", [128, N], BF16, kind="ExternalInput")
    era5a = nc.dram_tensor("era5a", [128, N], BF16, kind="ExternalInput")
    era5b = nc.dram_tensor("era5b", [128, N], BF16, kind="ExternalInput")
    wq_t = nc.dram_tensor("wq_t", [128, 128], BF16, kind="ExternalInput")
    wk_t0 = nc.dram_tensor("wk_t0", [128, 128], BF16, kind="ExternalInput")
    wk_t1 = nc.dram_tensor("wk_t1", [128, 128], BF16, kind="ExternalInput")
    wp_t0 = nc.dram_tensor("wp_t0", [128, 128], BF16, kind="ExternalInput")
    wp_t1 = nc.dram_tensor("wp_t1", [128, 128], BF16, kind="ExternalInput")
    bq_d = nc.dram_tensor("bq", [128, 1], F32, kind="ExternalInput")
    # output is stored TRANSPOSED: [N, 128] = (out + bias)^T without bias;
    # host adds the (folded) bias and transposes back.
    out_d = nc.dram_tensor("out", [N, 128], F32, kind="ExternalOutput")

    with tile.TileContext(nc) as tc, ExitStack() as ctx:
        consts = ctx.enter_context(tc.tile_pool(name="consts", bufs=1))
        big = ctx.enter_context(tc.tile_pool(name="big", bufs=1))
        ppool = ctx.enter_context(tc.tile_pool(name="pn", bufs=2))
        opool = ctx.enter_context(tc.tile_pool(name="small", bufs=2))
        ps_s = ctx.enter_context(tc.tile_pool(name="ps_s", bufs=2, space="PSUM"))
        ps_o = ctx.enter_context(tc.tile_pool(name="ps_o", bufs=2, space="PSUM"))

        # ---- constants / weights to SBUF ----
        wq_sb = consts.tile([128, 128], BF16, tag="wq")
        wk0_sb = consts.tile([128, 128], BF16, tag="wk0")
        wk1_sb = consts.tile([128, 128], BF16, tag="wk1")
        wp0_sb = consts.tile([128, 128], BF16, tag="wp0")
        wp1_sb = consts.tile([128, 128], BF16, tag="wp1")
        bq_sb = consts.tile([128, 1], F32, tag="bq")
        nc.sync.dma_start(wq_sb[:], wq_t[:])
        nc.sync.dma_start(wk0_sb[:], wk_t0[:])
        nc.sync.dma_start(wk1_sb[:], wk_t1[:])
        nc.sync.dma_start(wp0_sb[:], wp_t0[:])
        nc.sync.dma_start(wp1_sb[:], wp_t1[:])
        nc.sync.dma_start(bq_sb[:], bq_d[:])

        # inputs as independent column-piece tiles => fine-grained DMA deps
        era5a_p = [big.tile([128, 1024], BF16, tag=f"e5a{i}", name=f"e5a{i}")
                   for i in range(4)]
        era5b_p = [big.tile([128, 1024], BF16, tag=f"e5b{i}", name=f"e5b{i}")
                   for i in range(4)]
        cape_p = [big.tile([128, 1024], BF16, tag=f"cp{i}", name=f"cp{i}")
                  for i in range(4)]
        for j in range(4):
            sl = slice(j * 1024, (j + 1) * 1024)
            nc.sync.dma_start(era5a_p[j][:], era5a[:, sl])
            nc.sync.dma_start(era5b_p[j][:], era5b[:, sl])
            nc.sync.dma_start(cape_p[j][:], cape[:, sl])

        q_sb = big.tile([128, N], BF16, tag="q")
        k_sb = big.tile([128, N], BF16, tag="k")
        vT_sb = big.tile([128, NKC * VSTride], BF16, tag="vT")

        # ---- projections ----
        # k = Wk @ era5   [D, N]  (first: the S matmuls need all of k; the
        # PSUM->SBUF copies are split across ScalarE/VectorE)
        for j in range(4):
            pk = ps_s.tile([128, 1024], F32, tag="s", name=f"pk{j}")
            for h in range(2):
                osl = slice(h * 512, (h + 1) * 512)
                nc.tensor.matmul(pk[:, osl], wk0_sb[:], era5a_p[j][:, osl],
                                 start=True, stop=False)
                nc.tensor.matmul(pk[:, osl], wk1_sb[:], era5b_p[j][:, osl],
                                 start=False, stop=True)
            ksl = slice(j * 1024, (j + 1) * 1024)
            if j % 2 == 0:
                nc.scalar.activation(k_sb[:, ksl], pk[:], AFT.Copy)
            else:
                nc.vector.tensor_copy(k_sb[:, ksl], pk[:])
        # q block 0 only — it gates the first exp; the rest comes in s0's slots
        pq0 = ps_s.tile([128, 512], F32, tag="s", name="pq0")
        nc.tensor.matmul(pq0[:], wq_sb[:], cape_p[0][:, 0:512])
        nc.vector.tensor_scalar_add(q_sb[:, 0:512], pq0[:], bq_sb[:])

        def emit_q(j):          # q block j (512 cols), via a ps_o bank
            pq = ps_o.tile([128, 512], F32, tag="o", name=f"pq{j}")
            p, off = j // 2, (j % 2) * 512
            nc.tensor.matmul(pq[:], wq_sb[:], cape_p[p][:, off:off + 512])
            nc.vector.tensor_scalar_add(q_sb[:, j * 512:(j + 1) * 512],
                                        pq[:], bq_sb[:])

        # v'T chunks (v'T[kk, d] = era5^T @ (Wo Wv)^T), generated inside
        # superblock 0's group slots through the then-idle ps_o banks.
        vT_view = vT_sb.rearrange("p (c x) -> p c x", x=VSTride)

        def emit_vt_group(c4):
            pv = ps_o.tile([128, 512], F32, tag="o", name=f"pv_{c4}")
            for i in range(4):
                c = c4 * 4 + i
                p, off = c // 8, (c % 8) * 128
                osl = slice(i * 128, (i + 1) * 128)
                nc.tensor.matmul(pv[:, osl], era5a_p[p][:, off:off + 128],
                                 wp0_sb[:], start=True, stop=False)
                nc.tensor.matmul(pv[:, osl], era5b_p[p][:, off:off + 128],
                                 wp1_sb[:], start=False, stop=True)
            nc.vector.tensor_copy(
                vT_view[:, c4 * 4:(c4 + 1) * 4, 0:128],
                pv[:].rearrange("p (c x) -> p c x", x=128))
        # ones column (softmax denominator) per v'T chunk
        nc.gpsimd.memset(vT_view[:, :, 128:129], 1.0)

        # ---- main attention loop over query superblocks of 512 ----
        # Software-pipelined: superblock s runs S^T+exp while PE also runs the
        # value matmuls (VP) of superblock s-1 from its staged P buffer.
        p_bufs = {}       # s -> [128, 8192] bf16 staged exp(S^T)
        o_tiles = {}      # (s, j) -> [128, 129] psum accumulator

        def emit_vp_group(s, j, c_lo, c_hi):
            """VP matmuls for superblock s, query sub-block j, chunks [c_lo, c_hi)."""
            o_t = o_tiles[(s, j)]
            p_b = p_bufs[s]
            for c in range(c_lo, c_hi):
                lhs = p_b[:, c * 512 + j * 128: c * 512 + j * 128 + 128]
                nc.tensor.matmul(o_t[:], lhs,
                                 vT_sb[:, c * VSTride:c * VSTride + 129],
                                 start=(c == 0), stop=(c == NKC - 1))

        def emit_post(s, j):
            o_t = o_tiles.pop((s, j))
            recip_t = opool.tile([128, 1], F32, tag="recip")
            nc.vector.reciprocal(recip_t[:], o_t[:, 128:129])
            nrm_t = opool.tile([128, 128], F32, tag="nrm")
            nc.vector.tensor_scalar_mul(nrm_t[:], o_t[:, 0:128], recip_t[:])
            row = s * 512 + j * 128
            nc.sync.dma_start(out_d[row:row + 128, :], nrm_t[:])

        # VP work for superblock s-1 is spread over the 11 exp-group slots of
        # superblock s, j-major so at most 2 o_tiles are live.
        vp_sched = []     # per group-slot: list of (j, c_lo, c_hi)
        per_slot = (4 * NKC) // len(GROUPS) + 1   # ~12 chunk-MMs per slot
        flat = [(j, c) for j in range(4) for c in range(NKC)]
        for gi in range(len(GROUPS)):
            chunk = flat[gi * per_slot:(gi + 1) * per_slot]
            sched = []
            for (j, c) in chunk:
                if sched and sched[-1][0] == j and sched[-1][2] == c:
                    sched[-1] = (j, sched[-1][1], c + 1)
                else:
                    sched.append((j, c, c + 1))
            vp_sched.append(sched)

        for s in range(NSB):
            qsl = slice(s * 512, (s + 1) * 512)
            p_b = ppool.tile([128, NKC * 512], BF16, tag="p")
            p_bufs[s] = p_b
            c0 = 0
            for gi, G in enumerate(GROUPS):
                s_tile = ps_s.tile([128, G * 512], F32, tag="s")
                for i in range(G):
                    c = c0 + i
                    nc.tensor.matmul(s_tile[:, i * 512:(i + 1) * 512],
                                     k_sb[:, c * 128:(c + 1) * 128],
                                     q_sb[:, qsl])
                nc.scalar.activation(
                    p_b[:, c0 * 512:(c0 + G) * 512], s_tile[:], AFT.Exp)
                c0 += G
                # interleave previous superblock's VP + posts (superblock 0
                # interleaves the v'T generation instead)
                if s > 0:
                    for (j, c_lo, c_hi) in vp_sched[gi]:
                        if c_lo == 0:
                            o_tiles[(s - 1, j)] = ps_o.tile([128, 129], F32, tag="o", name=f"o_{s-1}_{j}")
                        emit_vp_group(s - 1, j, c_lo, c_hi)
                        if c_hi == NKC:
                            emit_post(s - 1, j)
                else:
                    if gi < NKC // 4:
                        emit_vt_group(gi)
                    if 1 + gi < 8:
                        emit_q(1 + gi)
            if s > 0:
                p_bufs.pop(s - 1)

        # pipeline tail: VP + post of the last superblock
        s = NSB - 1
        for j in range(4):
            o_tiles[(s, j)] = ps_o.tile([128, 129], F32, tag="o", name=f"o_{s}_{j}")
            emit_vp_group(s, j, 0, NKC)
            emit_post(s, j)

    nc.compile()
    return nc


def _get_program():
    if "nc" not in _CACHE:
        _CACHE["nc"] = build_program()
    return _CACHE["nc"]


def kernel(cape_features, era5_features, Wq, bq, Wk, bk, Wv, bv, Wo, bo):
    global LAST_RESULTS
    bf = ml_dtypes.bfloat16
    cape = np.asarray(cape_features, np.float32)
    era5 = np.asarray(era5_features, np.float32)
    Wq = np.asarray(Wq, np.float32)
    bq = np.asarray(bq, np.float32)
    Wk = np.asarray(Wk, np.float32)
    Wv = np.asarray(Wv, np.float32)
    bv = np.asarray(bv, np.float32)
    Wo = np.asarray(Wo, np.float32)
    bo = np.asarray(bo, np.float32)

    B = cape.shape[0]
    scale = np.float32(Wq.shape[0] ** -0.5)

    wq_t = np.ascontiguousarray((Wq * scale).T).astype(bf)       # [Cc, D]
    wk_t = np.ascontiguousarray(Wk.T)                            # [Ce, D]
    Wp = Wo @ Wv                                                 # [Cc, Ce]
    wp_t = np.ascontiguousarray(Wp.T)                            # [Ce, Cc]
    bq_e = np.ascontiguousarray((bq * scale).reshape(128, 1), dtype=np.float32)
    bp_e = (Wo @ bv + bo).astype(np.float32)          # added host-side

    common = {
        "wq_t": wq_t,
        "wk_t0": wk_t[:128].astype(bf), "wk_t1": wk_t[128:].astype(bf),
        "wp_t0": wp_t[:128].astype(bf), "wp_t1": wp_t[128:].astype(bf),
        "bq": bq_e,
    }
    in_maps = []
    for s in range(B):
        e = era5[s].reshape(256, N)
        in_maps.append(dict(common,
                            cape=cape[s].reshape(128, N).astype(bf),
                            era5a=e[:128].astype(bf),
                            era5b=e[128:].astype(bf)))

    nc = _get_program()
    res = run_bass_kernel_spmd(
        nc, in_maps, core_ids=list(range(NCORES)),
        trace=bool(int(os.environ.get("KBENCH_TRACE", "0"))),
    )
    LAST_RESULTS = res
    out = np.stack([
        (res.results[s]["out"].T + bp_e[:, None]).reshape(128, 64, 64)
        for s in range(B)
    ])
    return np.ascontiguousarray(out, dtype=np.float32)


# revision 19
# speedup vs baseline: 1.2966x; 1.0031x over previous
"""Cross-modal attention kernel for Trainium2 (Bass/Tile), data-parallel over
batch across 8 NeuronCores.

Math (per batch sample, N = 64*64 = 4096, D = 128):
    q = (s*Wq) @ cape + s*bq          [D, N]   (s = D**-0.5 folded into Wq,bq)
    k = Wk @ era5                     [D, N]   (bk dropped: constant along the
                                               softmax axis, cancels)
    S^T = k^T q                       [N, N]   computed in [128kk x 128qq] tiles
    P = exp(S^T)                      softmax numerator, kk on partitions
    U = (Wo@Wv @ era5) @ P            [128, N] Wo folded into V; softmax
                                               denominator = ones-column of the
                                               rhs -> column 128 of the output
    out = U[:, :128]/denom + (Wo@bv + bo)

Normalization is deferred past the value/output projections (both linear per
query column), so no per-element multiply over the NxN attention matrix is
ever needed; the denominator rides along as a 129th matmul output column.
"""

import os
import numpy as np
from contextlib import ExitStack

import concourse.bass as bass
import concourse.bacc as bacc
import concourse.mybir as mybir
import concourse.tile as tile
from concourse.bass_utils import run_bass_kernel_spmd
import ml_dtypes

AFT = mybir.ActivationFunctionType
BF16 = mybir.dt.bfloat16
F32 = mybir.dt.float32

N = 4096          # h*w
D = 128           # attn dim == cape channels
NCORES = 8
NKC = N // 128    # 32 kk chunks of 128
NSB = N // 512    # 8 query superblocks of 512
GROUPS = (3, 3, 3, 3, 3, 3, 3, 3, 3, 3, 2)   # kk chunks per exp group
VSTride = 136     # free-dim stride of one v'T chunk in SBUF (128 data + ones + pad)

_CACHE = {}
LAST_RESULTS = None


def build_program():
    nc = bacc.Bacc("TRN2", debug=False, target_bir_lowering=False)

    cape = nc.dram_tensor("cape", [128, N], BF16, kind="ExternalInput")
    era5a = nc.dram_tensor("era5a", [128, N], BF16, kind="ExternalInput")
    era5b = nc.dram_tensor("era5b", [128, N], BF16, kind="ExternalInput")
    wq_t = nc.dram_tensor("wq_t", [128, 128], BF16, kind="ExternalInput")
    wk_t0 = nc.dram_tensor("wk_t0", [128, 128], BF16, kind="ExternalInput")
    wk_t1 = nc.dram_tensor("wk_t1", [128, 128], BF16, kind="ExternalInput")
    wp_t0 = nc.dram_tensor("wp_t0", [128, 128], BF16, kind="ExternalInput")
    wp_t1 = nc.dram_tensor("wp_t1", [128, 128], BF16, kind="ExternalInput")
    bq_d = nc.dram_tensor("bq", [128, 1], F32, kind="ExternalInput")
    # output is stored TRANSPOSED: [N, 128] = (out + bias)^T without bias;
    # host adds the (folded) bias and transposes back.
    out_d = nc.dram_tensor("out", [N, 128], F32, kind="ExternalOutput")

    with tile.TileContext(nc) as tc, ExitStack() as ctx:
        consts = ctx.enter_context(tc.tile_pool(name="consts", bufs=1))
        big = ctx.enter_context(tc.tile_pool(name="big", bufs=1))
        ppool = ctx.enter_context(tc.tile_pool(name="pn", bufs=2))
        opool = ctx.enter_context(tc.tile_pool(name="small", bufs=2))
        ps_s = ctx.enter_context(tc.tile_pool(name="ps_s", bufs=2, space="PSUM"))
        ps_o = ctx.enter_context(tc.tile_pool(name="ps_o", bufs=2, space="PSUM"))

        # ---- constants / weights to SBUF ----
        wq_sb = consts.tile([128, 128], BF16, tag="wq")
        wk0_sb = consts.tile([128, 128], BF16, tag="wk0")
        wk1_sb = consts.tile([128, 128], BF16, tag="wk1")
        wp0_sb = consts.tile([128, 128], BF16, tag="wp0")
        wp1_sb = consts.tile([128, 128], BF16, tag="wp1")
        bq_sb = consts.tile([128, 1], F32, tag="bq")
        nc.sync.dma_start(wq_sb[:], wq_t[:])
        nc.sync.dma_start(wk0_sb[:], wk_t0[:])
        nc.sync.dma_start(wk1_sb[:], wk_t1[:])
        nc.sync.dma_start(wp0_sb[:], wp_t0[:])
        nc.sync.dma_start(wp1_sb[:], wp_t1[:])
        nc.sync.dma_start(bq_sb[:], bq_d[:])

        # inputs as independent column-piece tiles => fine-grained DMA deps
        era5a_p = [big.tile([128, 1024], BF16, tag=f"e5a{i}", name=f"e5a{i}")
                   for i in range(4)]
        era5b_p = [big.tile([128, 1024], BF16, tag=f"e5b{i}", name=f"e5b{i}")
                   for i in range(4)]
        cape_p = [big.tile([128, 1024], BF16, tag=f"cp{i}", name=f"cp{i}")
                  for i in range(4)]
        # era5 gates k which gates the first exp — load it first; cape piece 0
        # is needed early (q block 0), the rest of cape can trail on the
        # gpsimd queues.
        for j in range(4):
            sl = slice(j * 1024, (j + 1) * 1024)
            nc.sync.dma_start(era5a_p[j][:], era5a[:, sl])
            nc.sync.dma_start(era5b_p[j][:], era5b[:, sl])
        nc.gpsimd.dma_start(cape_p[0][:], cape[:, 0:1024])
        for j in range(1, 4):
            sl = slice(j * 1024, (j + 1) * 1024)
            nc.gpsimd.dma_start(cape_p[j][:], cape[:, sl])

        q_sb = big.tile([128, N], BF16, tag="q")
        k_sb = big.tile([128, N], BF16, tag="k")
        vT_sb = big.tile([128, NKC * VSTride], BF16, tag="vT")

        # ---- projections ----
        # k = Wk @ era5   [D, N]  (first: the S matmuls need all of k; the
        # PSUM->SBUF copies are split across ScalarE/VectorE)
        for j in range(4):
            pk = ps_s.tile([128, 1024], F32, tag="s", name=f"pk{j}")
            for h in range(2):
                osl = slice(h * 512, (h + 1) * 512)
                nc.tensor.matmul(pk[:, osl], wk0_sb[:], era5a_p[j][:, osl],
                                 start=True, stop=False)
                nc.tensor.matmul(pk[:, osl], wk1_sb[:], era5b_p[j][:, osl],
                                 start=False, stop=True)
            ksl = slice(j * 1024, (j + 1) * 1024)
            if j % 2 == 0:
                nc.scalar.activation(k_sb[:, ksl], pk[:], AFT.Copy)
            else:
                nc.vector.tensor_copy(k_sb[:, ksl], pk[:])
        # q block 0 only — it gates the first exp; the rest comes in s0's slots
        pq0 = ps_s.tile([128, 512], F32, tag="s", name="pq0")
        nc.tensor.matmul(pq0[:], wq_sb[:], cape_p[0][:, 0:512])
        nc.vector.tensor_scalar_add(q_sb[:, 0:512], pq0[:], bq_sb[:])

        def emit_q(j):          # q block j (512 cols), via a ps_o bank
            pq = ps_o.tile([128, 512], F32, tag="o", name=f"pq{j}")
            p, off = j // 2, (j % 2) * 512
            nc.tensor.matmul(pq[:], wq_sb[:], cape_p[p][:, off:off + 512])
            nc.vector.tensor_scalar_add(q_sb[:, j * 512:(j + 1) * 512],
                                        pq[:], bq_sb[:])

        # v'T chunks (v'T[kk, d] = era5^T @ (Wo Wv)^T), generated inside
        # superblock 0's group slots through the then-idle ps_o banks.
        vT_view = vT_sb.rearrange("p (c x) -> p c x", x=VSTride)

        def emit_vt_group(c4):
            pv = ps_o.tile([128, 512], F32, tag="o", name=f"pv_{c4}")
            for i in range(4):
                c = c4 * 4 + i
                p, off = c // 8, (c % 8) * 128
                osl = slice(i * 128, (i + 1) * 128)
                nc.tensor.matmul(pv[:, osl], era5a_p[p][:, off:off + 128],
                                 wp0_sb[:], start=True, stop=False)
                nc.tensor.matmul(pv[:, osl], era5b_p[p][:, off:off + 128],
                                 wp1_sb[:], start=False, stop=True)
            nc.vector.tensor_copy(
                vT_view[:, c4 * 4:(c4 + 1) * 4, 0:128],
                pv[:].rearrange("p (c x) -> p c x", x=128))
        # ones column (softmax denominator) per v'T chunk
        nc.gpsimd.memset(vT_view[:, :, 128:129], 1.0)

        # ---- main attention loop over query superblocks of 512 ----
        # Software-pipelined: superblock s runs S^T+exp while PE also runs the
        # value matmuls (VP) of superblock s-1 from its staged P buffer.
        p_bufs = {}       # s -> [128, 8192] bf16 staged exp(S^T)
        o_tiles = {}      # (s, j) -> [128, 129] psum accumulator

        def emit_vp_group(s, j, c_lo, c_hi):
            """VP matmuls for superblock s, query sub-block j, chunks [c_lo, c_hi)."""
            o_t = o_tiles[(s, j)]
            p_b = p_bufs[s]
            for c in range(c_lo, c_hi):
                lhs = p_b[:, c * 512 + j * 128: c * 512 + j * 128 + 128]
                nc.tensor.matmul(o_t[:], lhs,
                                 vT_sb[:, c * VSTride:c * VSTride + 129],
                                 start=(c == 0), stop=(c == NKC - 1))

        def emit_post(s, j):
            o_t = o_tiles.pop((s, j))
            recip_t = opool.tile([128, 1], F32, tag="recip")
            nc.vector.reciprocal(recip_t[:], o_t[:, 128:129])
            nrm_t = opool.tile([128, 128], F32, tag="nrm")
            nc.vector.tensor_scalar_mul(nrm_t[:], o_t[:, 0:128], recip_t[:])
            row = s * 512 + j * 128
            nc.sync.dma_start(out_d[row:row + 128, :], nrm_t[:])

        # VP work for superblock s-1 is spread over the 11 exp-group slots of
        # superblock s, j-major so at most 2 o_tiles are live.
        vp_sched = []     # per group-slot: list of (j, c_lo, c_hi)
        per_slot = (4 * NKC) // len(GROUPS) + 1   # ~12 chunk-MMs per slot
        flat = [(j, c) for j in range(4) for c in range(NKC)]
        for gi in range(len(GROUPS)):
            chunk = flat[gi * per_slot:(gi + 1) * per_slot]
            sched = []
            for (j, c) in chunk:
                if sched and sched[-1][0] == j and sched[-1][2] == c:
                    sched[-1] = (j, sched[-1][1], c + 1)
                else:
                    sched.append((j, c, c + 1))
            vp_sched.append(sched)

        for s in range(NSB):
            qsl = slice(s * 512, (s + 1) * 512)
            p_b = ppool.tile([128, NKC * 512], BF16, tag="p")
            p_bufs[s] = p_b
            c0 = 0
            for gi, G in enumerate(GROUPS):
                s_tile = ps_s.tile([128, G * 512], F32, tag="s")
                for i in range(G):
                    c = c0 + i
                    nc.tensor.matmul(s_tile[:, i * 512:(i + 1) * 512],
                                     k_sb[:, c * 128:(c + 1) * 128],
                                     q_sb[:, qsl])
                nc.scalar.activation(
                    p_b[:, c0 * 512:(c0 + G) * 512], s_tile[:], AFT.Exp)
                c0 += G
                # interleave previous superblock's VP + posts (superblock 0
                # interleaves the v'T generation instead)
                if s > 0:
                    for (j, c_lo, c_hi) in vp_sched[gi]:
                        if c_lo == 0:
                            o_tiles[(s - 1, j)] = ps_o.tile([128, 129], F32, tag="o", name=f"o_{s-1}_{j}")
                        emit_vp_group(s - 1, j, c_lo, c_hi)
                        if c_hi == NKC:
                            emit_post(s - 1, j)
                else:
                    if gi < NKC // 4:
                        emit_vt_group(gi)
                    if 1 + gi < 8:
                        emit_q(1 + gi)
            if s > 0:
                p_bufs.pop(s - 1)

        # pipeline tail: VP + post of the last superblock
        s = NSB - 1
        for j in range(4):
            o_tiles[(s, j)] = ps_o.tile([128, 129], F32, tag="o", name=f"o_{s}_{j}")
            emit_vp_group(s, j, 0, NKC)
            emit_post(s, j)

    nc.compile()
    return nc


def _get_program():
    if "nc" not in _CACHE:
        _CACHE["nc"] = build_program()
    return _CACHE["nc"]


def kernel(cape_features, era5_features, Wq, bq, Wk, bk, Wv, bv, Wo, bo):
    global LAST_RESULTS
    bf = ml_dtypes.bfloat16
    cape = np.asarray(cape_features, np.float32)
    era5 = np.asarray(era5_features, np.float32)
    Wq = np.asarray(Wq, np.float32)
    bq = np.asarray(bq, np.float32)
    Wk = np.asarray(Wk, np.float32)
    Wv = np.asarray(Wv, np.float32)
    bv = np.asarray(bv, np.float32)
    Wo = np.asarray(Wo, np.float32)
    bo = np.asarray(bo, np.float32)

    B = cape.shape[0]
    scale = np.float32(Wq.shape[0] ** -0.5)

    wq_t = np.ascontiguousarray((Wq * scale).T).astype(bf)       # [Cc, D]
    wk_t = np.ascontiguousarray(Wk.T)                            # [Ce, D]
    Wp = Wo @ Wv                                                 # [Cc, Ce]
    wp_t = np.ascontiguousarray(Wp.T)                            # [Ce, Cc]
    bq_e = np.ascontiguousarray((bq * scale).reshape(128, 1), dtype=np.float32)
    bp_e = (Wo @ bv + bo).astype(np.float32)          # added host-side

    common = {
        "wq_t": wq_t,
        "wk_t0": wk_t[:128].astype(bf), "wk_t1": wk_t[128:].astype(bf),
        "wp_t0": wp_t[:128].astype(bf), "wp_t1": wp_t[128:].astype(bf),
        "bq": bq_e,
    }
    in_maps = []
    for s in range(B):
        e = era5[s].reshape(256, N)
        in_maps.append(dict(common,
                            cape=cape[s].reshape(128, N).astype(bf),
                            era5a=e[:128].astype(bf),
                            era5b=e[128:].astype(bf)))

    nc = _get_program()
    res = run_bass_kernel_spmd(
        nc, in_maps, core_ids=list(range(NCORES)),
        trace=bool(int(os.environ.get("KBENCH_TRACE", "0"))),
    )
    LAST_RESULTS = res
    out = np.stack([
        (res.results[s]["out"].T + bp_e[:, None]).reshape(128, 64, 64)
        for s in range(B)
    ])
    return np.ascontiguousarray(out, dtype=np.float32)


# revision 23
# speedup vs baseline: 1.3029x; 1.0049x over previous
"""Cross-modal attention kernel for Trainium2 (Bass/Tile), data-parallel over
batch across 8 NeuronCores.

Math (per batch sample, N = 64*64 = 4096, D = 128):
    q = (s*Wq) @ cape + s*bq          [D, N]   (s = D**-0.5 folded into Wq,bq)
    k = Wk @ era5                     [D, N]   (bk dropped: constant along the
                                               softmax axis, cancels)
    S^T = k^T q                       [N, N]   computed in [128kk x 128qq] tiles
    P = exp(S^T)                      softmax numerator, kk on partitions
    U = (Wo@Wv @ era5) @ P            [128, N] Wo folded into V; softmax
                                               denominator = ones-column of the
                                               rhs -> column 128 of the output
    out = U[:, :128]/denom + (Wo@bv + bo)

Normalization is deferred past the value/output projections (both linear per
query column), so no per-element multiply over the NxN attention matrix is
ever needed; the denominator rides along as a 129th matmul output column.
"""

import os
import numpy as np
from contextlib import ExitStack

import concourse.bass as bass
import concourse.bacc as bacc
import concourse.mybir as mybir
import concourse.tile as tile
from concourse.bass_utils import run_bass_kernel_spmd
import ml_dtypes

AFT = mybir.ActivationFunctionType
BF16 = mybir.dt.bfloat16
F32 = mybir.dt.float32

N = 4096          # h*w
D = 128           # attn dim == cape channels
NCORES = 8
NKC = N // 128    # 32 kk chunks of 128
NSB = N // 512    # 8 query superblocks of 512
GROUPS = (3, 3, 3, 3, 3, 3, 3, 3, 3, 3, 2)   # kk chunks per exp group
VSTride = 136     # free-dim stride of one v'T chunk in SBUF (128 data + ones + pad)

_CACHE = {}
LAST_RESULTS = None


def build_program():
    nc = bacc.Bacc("TRN2", debug=False, target_bir_lowering=False)

    cape = nc.dram_tensor("cape", [128, N], BF16, kind="ExternalInput")
    era5a = nc.dram_tensor("era5a", [128, N], BF16, kind="ExternalInput")
    era5b = nc.dram_tensor("era5b", [128, N], BF16, kind="ExternalInput")
    # all weights in one tensor (each dma_start costs ~650ns of sequencer
    # issue time — minimize DMA count): [wq_t|wk_t0|wk_t1|wp_t0|wp_t1|bq(f32
    # bitcast as 2 bf16 cols)]
    wpack_d = nc.dram_tensor("wpack", [128, 642], BF16, kind="ExternalInput")
    # output is stored TRANSPOSED: [N, 128] = (out + bias)^T without bias;
    # host adds the (folded) bias and transposes back.
    out_d = nc.dram_tensor("out", [N, 128], F32, kind="ExternalOutput")

    with tile.TileContext(nc) as tc, ExitStack() as ctx:
        consts = ctx.enter_context(tc.tile_pool(name="consts", bufs=1))
        big = ctx.enter_context(tc.tile_pool(name="big", bufs=1))
        ppool = ctx.enter_context(tc.tile_pool(name="pn", bufs=2))
        opool = ctx.enter_context(tc.tile_pool(name="small", bufs=2))
        ps_s = ctx.enter_context(tc.tile_pool(name="ps_s", bufs=2, space="PSUM"))
        ps_o = ctx.enter_context(tc.tile_pool(name="ps_o", bufs=2, space="PSUM"))

        # ---- constants / weights to SBUF (one DMA) ----
        wpack_sb = consts.tile([128, 642], BF16, tag="wpack")
        nc.sync.dma_start(wpack_sb[:], wpack_d[:])
        wq_sb = wpack_sb[:, 0:128]
        wk0_sb = wpack_sb[:, 128:256]
        wk1_sb = wpack_sb[:, 256:384]
        wp0_sb = wpack_sb[:, 384:512]
        wp1_sb = wpack_sb[:, 512:640]
        bq_sb = wpack_sb[:, 640:642].bitcast(F32)

        # whole-tensor input loads (era5 first: it gates k -> first exp)
        era5a_sb = big.tile([128, N], BF16, tag="era5a")
        era5b_sb = big.tile([128, N], BF16, tag="era5b")
        cape_sb = big.tile([128, N], BF16, tag="cape")
        nc.sync.dma_start(era5a_sb[:], era5a[:])
        nc.sync.dma_start(era5b_sb[:], era5b[:])
        nc.sync.dma_start(cape_sb[:], cape[:])

        q_sb = big.tile([128, N], BF16, tag="q")
        k_sb = big.tile([128, N], BF16, tag="k")
        vT_sb = big.tile([128, NKC * VSTride], BF16, tag="vT")

        # ---- projections ----
        # k = Wk @ era5   [D, N]  (first: the S matmuls need all of k; the
        # PSUM->SBUF copies are split across ScalarE/VectorE)
        for j in range(4):
            pk = ps_s.tile([128, 1024], F32, tag="s", name=f"pk{j}")
            for h in range(2):
                osl = slice(h * 512, (h + 1) * 512)
                isl = slice(j * 1024 + h * 512, j * 1024 + (h + 1) * 512)
                nc.tensor.matmul(pk[:, osl], wk0_sb, era5a_sb[:, isl],
                                 start=True, stop=False)
                nc.tensor.matmul(pk[:, osl], wk1_sb, era5b_sb[:, isl],
                                 start=False, stop=True)
            ksl = slice(j * 1024, (j + 1) * 1024)
            if j % 2 == 0:
                nc.scalar.activation(k_sb[:, ksl], pk[:], AFT.Copy)
            else:
                nc.vector.tensor_copy(k_sb[:, ksl], pk[:])
        # q block 0 only — it gates the first exp; the rest comes in s0's slots
        pq0 = ps_s.tile([128, 512], F32, tag="s", name="pq0")
        nc.tensor.matmul(pq0[:], wq_sb, cape_sb[:, 0:512])
        nc.vector.tensor_scalar_add(q_sb[:, 0:512], pq0[:], bq_sb)

        def emit_q(j):          # q block j (512 cols), via a ps_o bank
            pq = ps_o.tile([128, 512], F32, tag="o", name=f"pq{j}")
            sl = slice(j * 512, (j + 1) * 512)
            nc.tensor.matmul(pq[:], wq_sb, cape_sb[:, sl])
            nc.vector.tensor_scalar_add(q_sb[:, sl], pq[:], bq_sb)

        # v'T chunks (v'T[kk, d] = era5^T @ (Wo Wv)^T), generated inside
        # superblock 0's group slots through the then-idle ps_o banks.
        vT_view = vT_sb.rearrange("p (c x) -> p c x", x=VSTride)

        def emit_vt_group(c4):
            pv = ps_o.tile([128, 512], F32, tag="o", name=f"pv_{c4}")
            for i in range(4):
                c = c4 * 4 + i
                ksl = slice(c * 128, (c + 1) * 128)
                osl = slice(i * 128, (i + 1) * 128)
                nc.tensor.matmul(pv[:, osl], era5a_sb[:, ksl],
                                 wp0_sb, start=True, stop=False)
                nc.tensor.matmul(pv[:, osl], era5b_sb[:, ksl],
                                 wp1_sb, start=False, stop=True)
            nc.vector.tensor_copy(
                vT_view[:, c4 * 4:(c4 + 1) * 4, 0:128],
                pv[:].rearrange("p (c x) -> p c x", x=128))
        # ones column (softmax denominator) per v'T chunk
        nc.gpsimd.memset(vT_view[:, :, 128:129], 1.0)

        # ---- main attention loop over query superblocks of 512 ----
        # Software-pipelined: superblock s runs S^T+exp while PE also runs the
        # value matmuls (VP) of superblock s-1 from its staged P buffer.
        p_bufs = {}       # s -> [128, 8192] bf16 staged exp(S^T)
        o_tiles = {}      # (s, j) -> [128, 129] psum accumulator

        def emit_vp_group(s, j, c_lo, c_hi):
            """VP matmuls for superblock s, query sub-block j, chunks [c_lo, c_hi)."""
            o_t = o_tiles[(s, j)]
            p_b = p_bufs[s]
            for c in range(c_lo, c_hi):
                lhs = p_b[:, c * 512 + j * 128: c * 512 + j * 128 + 128]
                nc.tensor.matmul(o_t[:], lhs,
                                 vT_sb[:, c * VSTride:c * VSTride + 129],
                                 start=(c == 0), stop=(c == NKC - 1))

        def emit_post(s, j):
            o_t = o_tiles.pop((s, j))
            recip_t = opool.tile([128, 1], F32, tag="recip")
            nc.vector.reciprocal(recip_t[:], o_t[:, 128:129])
            nrm_t = opool.tile([128, 128], F32, tag="nrm")
            nc.vector.tensor_scalar_mul(nrm_t[:], o_t[:, 0:128], recip_t[:])
            row = s * 512 + j * 128
            nc.sync.dma_start(out_d[row:row + 128, :], nrm_t[:])

        # VP work for superblock s-1 is spread over the 11 exp-group slots of
        # superblock s, j-major so at most 2 o_tiles are live.
        vp_sched = []     # per group-slot: list of (j, c_lo, c_hi)
        per_slot = (4 * NKC) // len(GROUPS) + 1   # ~12 chunk-MMs per slot
        flat = [(j, c) for j in range(4) for c in range(NKC)]
        for gi in range(len(GROUPS)):
            chunk = flat[gi * per_slot:(gi + 1) * per_slot]
            sched = []
            for (j, c) in chunk:
                if sched and sched[-1][0] == j and sched[-1][2] == c:
                    sched[-1] = (j, sched[-1][1], c + 1)
                else:
                    sched.append((j, c, c + 1))
            vp_sched.append(sched)

        for s in range(NSB):
            qsl = slice(s * 512, (s + 1) * 512)
            p_b = ppool.tile([128, NKC * 512], BF16, tag="p")
            p_bufs[s] = p_b
            c0 = 0
            for gi, G in enumerate(GROUPS):
                s_tile = ps_s.tile([128, G * 512], F32, tag="s")
                for i in range(G):
                    c = c0 + i
                    nc.tensor.matmul(s_tile[:, i * 512:(i + 1) * 512],
                                     k_sb[:, c * 128:(c + 1) * 128],
                                     q_sb[:, qsl])
                nc.scalar.activation(
                    p_b[:, c0 * 512:(c0 + G) * 512], s_tile[:], AFT.Exp)
                c0 += G
                # interleave previous superblock's VP + posts (superblock 0
                # interleaves the v'T generation instead)
                if s > 0:
                    for (j, c_lo, c_hi) in vp_sched[gi]:
                        if c_lo == 0:
                            o_tiles[(s - 1, j)] = ps_o.tile([128, 129], F32, tag="o", name=f"o_{s-1}_{j}")
                        emit_vp_group(s - 1, j, c_lo, c_hi)
                        if c_hi == NKC:
                            emit_post(s - 1, j)
                else:
                    if gi < NKC // 4:
                        emit_vt_group(gi)
                    if 1 + gi < 8:
                        emit_q(1 + gi)
            if s > 0:
                p_bufs.pop(s - 1)

        # pipeline tail: VP + post of the last superblock
        s = NSB - 1
        for j in range(4):
            o_tiles[(s, j)] = ps_o.tile([128, 129], F32, tag="o", name=f"o_{s}_{j}")
            emit_vp_group(s, j, 0, NKC)
            emit_post(s, j)

    nc.compile()
    return nc


def _get_program():
    if "nc" not in _CACHE:
        _CACHE["nc"] = build_program()
    return _CACHE["nc"]


def kernel(cape_features, era5_features, Wq, bq, Wk, bk, Wv, bv, Wo, bo):
    global LAST_RESULTS
    bf = ml_dtypes.bfloat16
    cape = np.asarray(cape_features, np.float32)
    era5 = np.asarray(era5_features, np.float32)
    Wq = np.asarray(Wq, np.float32)
    bq = np.asarray(bq, np.float32)
    Wk = np.asarray(Wk, np.float32)
    Wv = np.asarray(Wv, np.float32)
    bv = np.asarray(bv, np.float32)
    Wo = np.asarray(Wo, np.float32)
    bo = np.asarray(bo, np.float32)

    B = cape.shape[0]
    scale = np.float32(Wq.shape[0] ** -0.5)

    wq_t = np.ascontiguousarray((Wq * scale).T).astype(bf)       # [Cc, D]
    wk_t = np.ascontiguousarray(Wk.T)                            # [Ce, D]
    Wp = Wo @ Wv                                                 # [Cc, Ce]
    wp_t = np.ascontiguousarray(Wp.T)                            # [Ce, Cc]
    bq_e = np.ascontiguousarray((bq * scale).reshape(128, 1), dtype=np.float32)
    bp_e = (Wo @ bv + bo).astype(np.float32)          # added host-side

    wpack = np.zeros((128, 642), dtype=bf)
    wpack[:, 0:128] = wq_t
    wpack[:, 128:256] = wk_t[:128].astype(bf)
    wpack[:, 256:384] = wk_t[128:].astype(bf)
    wpack[:, 384:512] = wp_t[:128].astype(bf)
    wpack[:, 512:640] = wp_t[128:].astype(bf)
    wpack[:, 640:642] = bq_e.view(bf)                 # f32 bits as 2 bf16 cols
    common = {"wpack": wpack}
    in_maps = []
    for s in range(B):
        e = era5[s].reshape(256, N)
        in_maps.append(dict(common,
                            cape=cape[s].reshape(128, N).astype(bf),
                            era5a=e[:128].astype(bf),
                            era5b=e[128:].astype(bf)))

    nc = _get_program()
    res = run_bass_kernel_spmd(
        nc, in_maps, core_ids=list(range(NCORES)),
        trace=bool(int(os.environ.get("KBENCH_TRACE", "0"))),
    )
    LAST_RESULTS = res
    out = np.stack([
        (res.results[s]["out"].T + bp_e[:, None]).reshape(128, 64, 64)
        for s in range(B)
    ])
    return np.ascontiguousarray(out, dtype=np.float32)


# revision 27
# speedup vs baseline: 1.3340x; 1.0239x over previous
"""Cross-modal attention kernel for Trainium2 (Bass/Tile), data-parallel over
batch across 8 NeuronCores.

Math (per batch sample, N = 64*64 = 4096, D = 128):
    q = (s*Wq) @ cape + s*bq          [D, N]   (s = D**-0.5 folded into Wq,bq)
    k = Wk @ era5                     [D, N]   (bk dropped: constant along the
                                               softmax axis, cancels)
    S^T = k^T q                       [N, N]   computed in [128kk x 128qq] tiles
    P = exp(S^T)                      softmax numerator, kk on partitions
    U = (Wo@Wv @ era5) @ P            [128, N] Wo folded into V; softmax
                                               denominator = ones-column of the
                                               rhs -> column 128 of the output
    out = U[:, :128]/denom + (Wo@bv + bo)

Normalization is deferred past the value/output projections (both linear per
query column), so no per-element multiply over the NxN attention matrix is
ever needed; the denominator rides along as a 129th matmul output column.
"""

import os
import numpy as np
from contextlib import ExitStack

import concourse.bass as bass
import concourse.bacc as bacc
import concourse.mybir as mybir
import concourse.tile as tile
from concourse.bass_utils import run_bass_kernel_spmd
import ml_dtypes

AFT = mybir.ActivationFunctionType
BF16 = mybir.dt.bfloat16
F32 = mybir.dt.float32

N = 4096          # h*w
D = 128           # attn dim == cape channels
NCORES = 8
NKC = N // 128    # 32 kk chunks of 128
NSB = N // 512    # 8 query superblocks of 512
GROUPS = (3, 3, 3, 3, 3, 3, 3, 3, 3, 3, 2)   # kk chunks per exp group
VSTride = 136     # free-dim stride of one v'T chunk in SBUF (128 data + ones + pad)

_CACHE = {}
LAST_RESULTS = None


def build_program():
    nc = bacc.Bacc("TRN2", debug=False, target_bir_lowering=False)

    cape = nc.dram_tensor("cape", [128, N], BF16, kind="ExternalInput")
    era5a = nc.dram_tensor("era5a", [128, N], BF16, kind="ExternalInput")
    era5b = nc.dram_tensor("era5b", [128, N], BF16, kind="ExternalInput")
    # all weights in one tensor (each dma_start costs ~650ns of sequencer
    # issue time — minimize DMA count): [wq_t|wk_t0|wk_t1|wp_t0|wp_t1|bq(f32
    # bitcast as 2 bf16 cols)]
    wpack_d = nc.dram_tensor("wpack", [128, 642], BF16, kind="ExternalInput")
    # output is stored TRANSPOSED: [N, 128] = (out + bias)^T without bias;
    # host adds the (folded) bias and transposes back.
    out_d = nc.dram_tensor("out", [N, 128], F32, kind="ExternalOutput")

    with tile.TileContext(nc) as tc, ExitStack() as ctx:
        consts = ctx.enter_context(tc.tile_pool(name="consts", bufs=1))
        big = ctx.enter_context(tc.tile_pool(name="big", bufs=1))
        ppool = ctx.enter_context(tc.tile_pool(name="pn", bufs=2))
        opool = ctx.enter_context(tc.tile_pool(name="small", bufs=2))
        ps_s = ctx.enter_context(tc.tile_pool(name="ps_s", bufs=2, space="PSUM"))
        ps_o = ctx.enter_context(tc.tile_pool(name="ps_o", bufs=2, space="PSUM"))

        # ---- constants / weights to SBUF (one DMA) ----
        wpack_sb = consts.tile([128, 642], BF16, tag="wpack")
        nc.sync.dma_start(wpack_sb[:], wpack_d[:])
        wq_sb = wpack_sb[:, 0:128]
        wk0_sb = wpack_sb[:, 128:256]
        wk1_sb = wpack_sb[:, 256:384]
        wp0_sb = wpack_sb[:, 384:512]
        wp1_sb = wpack_sb[:, 512:640]
        bq_sb = wpack_sb[:, 640:642].bitcast(F32)

        # input loads in arrival-priority order: era5 piece 0 gates the first k
        # tile; cape cols 0:512 gate q block 0; the rest streams underneath the
        # running pipeline.
        era5a_sb = big.tile([128, N], BF16, tag="era5a")
        era5b_sb = big.tile([128, N], BF16, tag="era5b")
        cape_sb = big.tile([128, N], BF16, tag="cape")
        EPIECES = ((0, 1536), (1536, 3072), (3072, 4096))
        nc.sync.dma_start(era5a_sb[:, 0:1536], era5a[:, 0:1536])
        nc.sync.dma_start(era5b_sb[:, 0:1536], era5b[:, 0:1536])
        nc.sync.dma_start(cape_sb[:, 0:512], cape[:, 0:512])
        for lo, hi in EPIECES[1:]:
            nc.sync.dma_start(era5a_sb[:, lo:hi], era5a[:, lo:hi])
            nc.sync.dma_start(era5b_sb[:, lo:hi], era5b[:, lo:hi])
        nc.sync.dma_start(cape_sb[:, 512:N], cape[:, 512:N])

        # PE pre-warm: ~4us of dummy matmuls on the (tiny, already-loaded)
        # weight tile flips the HAM clock gate to 2.4 GHz before real work
        # arrives (the cold-rate window would otherwise eat the whole head).
        warm = ps_o.tile([128, 512], F32, tag="o", name="warm")
        for _ in range(10):
            nc.tensor.matmul(warm[:], wq_sb, wpack_sb[:, 0:512])

        q_sb = big.tile([128, N], BF16, tag="q")
        k_sb = big.tile([128, N], BF16, tag="k")
        vT_sb = big.tile([128, NKC * VSTride], BF16, tag="vT")

        # ---- projections ----
        # k = Wk @ era5   [D, N]  (first: the S matmuls need all of k; the
        # PSUM->SBUF copies are split across ScalarE/VectorE)
        def emit_k_tile(t):
            lo, hi = EPIECES[t]
            w = hi - lo
            pk = ps_s.tile([128, w], F32, tag="s", name=f"pk{t}")
            for h in range(w // 512):
                osl = slice(h * 512, (h + 1) * 512)
                isl = slice(lo + h * 512, lo + (h + 1) * 512)
                nc.tensor.matmul(pk[:, osl], wk0_sb, era5a_sb[:, isl],
                                 start=True, stop=False)
                nc.tensor.matmul(pk[:, osl], wk1_sb, era5b_sb[:, isl],
                                 start=False, stop=True)
            if t == 0:
                # split so S(s0, g0) can start after the first 512 columns
                nc.scalar.activation(k_sb[:, 0:512], pk[:, 0:512], AFT.Copy)
                nc.vector.tensor_copy(k_sb[:, 512:1536], pk[:, 512:1536])
            elif t == 1:
                nc.scalar.activation(k_sb[:, lo:hi], pk[:], AFT.Copy)
            else:
                nc.vector.tensor_copy(k_sb[:, lo:hi], pk[:])

        # k tile 0 + q block 0 gate the first exp; k tiles 1-2 and the rest of
        # q are produced inside superblock 0's group slots.
        emit_k_tile(0)
        pq0 = ps_s.tile([128, 512], F32, tag="s", name="pq0")
        nc.tensor.matmul(pq0[:], wq_sb, cape_sb[:, 0:512])
        nc.vector.tensor_scalar_add(q_sb[:, 0:512], pq0[:], bq_sb)

        def emit_q(j):          # q block j (512 cols), via a ps_o bank
            pq = ps_o.tile([128, 512], F32, tag="o", name=f"pq{j}")
            sl = slice(j * 512, (j + 1) * 512)
            nc.tensor.matmul(pq[:], wq_sb, cape_sb[:, sl])
            nc.vector.tensor_scalar_add(q_sb[:, sl], pq[:], bq_sb)

        # v'T chunks (v'T[kk, d] = era5^T @ (Wo Wv)^T), generated inside
        # superblock 0's group slots through the then-idle ps_o banks.
        vT_view = vT_sb.rearrange("p (c x) -> p c x", x=VSTride)

        def emit_vt_group(c4):
            pv = ps_o.tile([128, 512], F32, tag="o", name=f"pv_{c4}")
            for i in range(4):
                c = c4 * 4 + i
                ksl = slice(c * 128, (c + 1) * 128)
                osl = slice(i * 128, (i + 1) * 128)
                nc.tensor.matmul(pv[:, osl], era5a_sb[:, ksl],
                                 wp0_sb, start=True, stop=False)
                nc.tensor.matmul(pv[:, osl], era5b_sb[:, ksl],
                                 wp1_sb, start=False, stop=True)
            nc.vector.tensor_copy(
                vT_view[:, c4 * 4:(c4 + 1) * 4, 0:128],
                pv[:].rearrange("p (c x) -> p c x", x=128))
        # ones column (softmax denominator) per v'T chunk
        nc.gpsimd.memset(vT_view[:, :, 128:129], 1.0)

        # ---- main attention loop over query superblocks of 512 ----
        # Software-pipelined: superblock s runs S^T+exp while PE also runs the
        # value matmuls (VP) of superblock s-1 from its staged P buffer.
        p_bufs = {}       # s -> [128, 8192] bf16 staged exp(S^T)
        o_tiles = {}      # (s, j) -> [128, 129] psum accumulator

        def emit_vp_group(s, j, c_lo, c_hi):
            """VP matmuls for superblock s, query sub-block j, chunks [c_lo, c_hi)."""
            o_t = o_tiles[(s, j)]
            p_b = p_bufs[s]
            for c in range(c_lo, c_hi):
                lhs = p_b[:, c * 512 + j * 128: c * 512 + j * 128 + 128]
                nc.tensor.matmul(o_t[:], lhs,
                                 vT_sb[:, c * VSTride:c * VSTride + 129],
                                 start=(c == 0), stop=(c == NKC - 1))

        def emit_post(s, j):
            o_t = o_tiles.pop((s, j))
            recip_t = opool.tile([128, 1], F32, tag="recip")
            nc.vector.reciprocal(recip_t[:], o_t[:, 128:129])
            nrm_t = opool.tile([128, 128], F32, tag="nrm")
            nc.vector.tensor_scalar_mul(nrm_t[:], o_t[:, 0:128], recip_t[:])
            row = s * 512 + j * 128
            nc.sync.dma_start(out_d[row:row + 128, :], nrm_t[:])

        # VP work for superblock s-1 is spread over the 11 exp-group slots of
        # superblock s, j-major so at most 2 o_tiles are live.
        vp_sched = []     # per group-slot: list of (j, c_lo, c_hi)
        per_slot = (4 * NKC) // len(GROUPS) + 1   # ~12 chunk-MMs per slot
        flat = [(j, c) for j in range(4) for c in range(NKC)]
        for gi in range(len(GROUPS)):
            chunk = flat[gi * per_slot:(gi + 1) * per_slot]
            sched = []
            for (j, c) in chunk:
                if sched and sched[-1][0] == j and sched[-1][2] == c:
                    sched[-1] = (j, sched[-1][1], c + 1)
                else:
                    sched.append((j, c, c + 1))
            vp_sched.append(sched)

        for s in range(NSB):
            qsl = slice(s * 512, (s + 1) * 512)
            p_b = ppool.tile([128, NKC * 512], BF16, tag="p")
            p_bufs[s] = p_b
            c0 = 0
            for gi, G in enumerate(GROUPS):
                s_tile = ps_s.tile([128, G * 512], F32, tag="s")
                for i in range(G):
                    c = c0 + i
                    nc.tensor.matmul(s_tile[:, i * 512:(i + 1) * 512],
                                     k_sb[:, c * 128:(c + 1) * 128],
                                     q_sb[:, qsl])
                nc.scalar.activation(
                    p_b[:, c0 * 512:(c0 + G) * 512], s_tile[:], AFT.Exp)
                c0 += G
                # interleave previous superblock's VP + posts (superblock 0
                # interleaves the v'T generation instead)
                if s > 0:
                    for (j, c_lo, c_hi) in vp_sched[gi]:
                        if c_lo == 0:
                            o_tiles[(s - 1, j)] = ps_o.tile([128, 129], F32, tag="o", name=f"o_{s-1}_{j}")
                        emit_vp_group(s - 1, j, c_lo, c_hi)
                        if c_hi == NKC:
                            emit_post(s - 1, j)
                else:
                    # s0 slot schedule: k tiles 1-2 arrive in time for the S
                    # groups that need them (g4 -> chunks 12+, g8 -> 24+);
                    # vT groups and the rest of q fill the other slots.
                    S0_SLOTS = {
                        0: [("vt", 0), ("q", 1)], 1: [("vt", 1), ("q", 2)],
                        2: [("k", 1), ("q", 3)], 3: [("vt", 2), ("q", 4)],
                        4: [("vt", 3), ("q", 5)], 5: [("vt", 4), ("q", 6)],
                        6: [("k", 2), ("q", 7)], 7: [("vt", 5)],
                        8: [("vt", 6)], 9: [("vt", 7)],
                    }
                    for kind, idx in S0_SLOTS.get(gi, []):
                        if kind == "vt":
                            emit_vt_group(idx)
                        elif kind == "q":
                            emit_q(idx)
                        else:
                            emit_k_tile(idx)
            if s > 0:
                p_bufs.pop(s - 1)

        # pipeline tail: VP + post of the last superblock
        s = NSB - 1
        for j in range(4):
            o_tiles[(s, j)] = ps_o.tile([128, 129], F32, tag="o", name=f"o_{s}_{j}")
            emit_vp_group(s, j, 0, NKC)
            emit_post(s, j)

    nc.compile()
    return nc


def _get_program():
    if "nc" not in _CACHE:
        _CACHE["nc"] = build_program()
    return _CACHE["nc"]


def kernel(cape_features, era5_features, Wq, bq, Wk, bk, Wv, bv, Wo, bo):
    global LAST_RESULTS
    bf = ml_dtypes.bfloat16
    cape = np.asarray(cape_features, np.float32)
    era5 = np.asarray(era5_features, np.float32)
    Wq = np.asarray(Wq, np.float32)
    bq = np.asarray(bq, np.float32)
    Wk = np.asarray(Wk, np.float32)
    Wv = np.asarray(Wv, np.float32)
    bv = np.asarray(bv, np.float32)
    Wo = np.asarray(Wo, np.float32)
    bo = np.asarray(bo, np.float32)

    B = cape.shape[0]
    scale = np.float32(Wq.shape[0] ** -0.5)

    wq_t = np.ascontiguousarray((Wq * scale).T).astype(bf)       # [Cc, D]
    wk_t = np.ascontiguousarray(Wk.T)                            # [Ce, D]
    Wp = Wo @ Wv                                                 # [Cc, Ce]
    wp_t = np.ascontiguousarray(Wp.T)                            # [Ce, Cc]
    bq_e = np.ascontiguousarray((bq * scale).reshape(128, 1), dtype=np.float32)
    bp_e = (Wo @ bv + bo).astype(np.float32)          # added host-side

    wpack = np.zeros((128, 642), dtype=bf)
    wpack[:, 0:128] = wq_t
    wpack[:, 128:256] = wk_t[:128].astype(bf)
    wpack[:, 256:384] = wk_t[128:].astype(bf)
    wpack[:, 384:512] = wp_t[:128].astype(bf)
    wpack[:, 512:640] = wp_t[128:].astype(bf)
    wpack[:, 640:642] = bq_e.view(bf)                 # f32 bits as 2 bf16 cols
    common = {"wpack": wpack}
    in_maps = []
    for s in range(B):
        e = era5[s].reshape(256, N)
        in_maps.append(dict(common,
                            cape=cape[s].reshape(128, N).astype(bf),
                            era5a=e[:128].astype(bf),
                            era5b=e[128:].astype(bf)))

    nc = _get_program()
    res = run_bass_kernel_spmd(
        nc, in_maps, core_ids=list(range(NCORES)),
        trace=bool(int(os.environ.get("KBENCH_TRACE", "0"))),
    )
    LAST_RESULTS = res
    out = np.stack([
        (res.results[s]["out"].T + bp_e[:, None]).reshape(128, 64, 64)
        for s in range(B)
    ])
    return np.ascontiguousarray(out, dtype=np.float32)
